# revision 26
# baseline (speedup 1.0000x reference)
"""Trainium2 Bass kernel: single attention head (B=8, S=2048, E=1024, H=64).

Sharding: data-parallel over batch -- each of the 8 NeuronCores computes one
batch element's full attention. No collectives; every HBM byte read once.

v2 design (streaming wavefront):
  - Inputs are cast to fp16 HOST-side and staged as [128, blk, chunk, 256]
    so every 512KB block DMA is one contiguous 4KB line per partition on the
    fast HWDGE (sync) queue. Halves HBM traffic vs f32 and frees GpSimd.
  - Q/K/V stream in 8 interleaved column-block rounds (q_b, k_b, v_b). The
    softmax exp -- the ScalarE floor at ~27us -- starts at ~3us and runs
    continuously instead of waiting for all projections.
  - Projections run as column-tiled concurrent pairs (2x PE): pass A puts
    q_b in BOTH partition halves (array cols 0-63 // 64-127); pass B puts
    k_b in half (b%2) and v_b in the other. This materializes q and k in
    both SBUF partition halves with zero cross-partition copies.
  - Scores are row-tiled 2x: even-parity key tiles use array rows 0-63
    (kt/qt low half), odd tiles rows 64-127, concurrently, into different
    PSUM banks. Scores stay transposed (keys on partitions) so softmax
    rowsums ride a ones-column in the AV stationary.
  - exp on ScalarE (scale=1/8 fused) writes fp16 P tiles; AV accumulates
    [v|1]^T @ P into one [65, 2048] PSUM right behind each exp cell.
  - Finalize: PE transposes 128-col chunks (PSUM regions reuse the proj
    bank via f16 bitcast), VectorE reciprocal + scale, batched f32 DMA out.

PSUM: 1 bank proj (A/B slices) + 3 banks score cells + 4 banks AV = 8.
"""

import numpy as np

import concourse.bass as bass  # noqa: F401  (engine namespaces live on nc)
import concourse.mybir as mybir
import concourse.tile as tile
from concourse import bacc
from concourse.bass_utils import run_bass_kernel_spmd
from concourse.masks import make_identity

B, S, E, H = 8, 2048, 1024, 64
EC = E // 128    # contraction chunks (128 partitions each)
NB = 8           # column-block rounds
CB = S // NB     # 256 columns per block
NT = S // 128    # key tiles
F16 = mybir.dt.float16
F32 = mybir.dt.float32

_CACHE = {}


def _build_nc():
    nc = bacc.Bacc(None)
    xq = nc.declare_dram_parameter("xq", [128, NB, EC, CB], F16, isOutput=False)
    xk = nc.declare_dram_parameter("xk", [128, NB, EC, CB], F16, isOutput=False)
    xv = nc.declare_dram_parameter("xv", [128, NB, EC, CB], F16, isOutput=False)
    wq = nc.declare_dram_parameter("wq", [128, EC, H], F16, isOutput=False)
    wk = nc.declare_dram_parameter("wk", [128, EC, H], F16, isOutput=False)
    wv = nc.declare_dram_parameter("wv", [128, EC, H], F16, isOutput=False)
    bq = nc.declare_dram_parameter("bq", [128, 1], F32, isOutput=False)
    bv = nc.declare_dram_parameter("bv", [128, 1], F32, isOutput=False)
    out = nc.declare_dram_parameter("out", [S, H], F32, isOutput=True)

    Exp = mybir.ActivationFunctionType.Exp

    with tile.TileContext(nc) as tc:
        with tc.tile_pool(name="const", bufs=1) as const, \
             tc.tile_pool(name="xqp", bufs=6) as xqp, \
             tc.tile_pool(name="xkp", bufs=6) as xkp, \
             tc.tile_pool(name="xvp", bufs=6) as xvp, \
             tc.tile_pool(name="vtp", bufs=2) as vtp, \
             tc.tile_pool(name="p5sb", bufs=2) as p5sb, \
             tc.tile_pool(name="pjp", bufs=1, space="PSUM") as pjp, \
             tc.tile_pool(name="scp", bufs=3, space="PSUM") as scp, \
             tc.tile_pool(name="oap", bufs=1, space="PSUM") as oap:

            # ---- constants ----
            wts = {}
            for nm, dram in (("q", wq), ("k", wk), ("v", wv)):
                wt = const.tile([128, EC, H], F16, name=f"w{nm}")
                nc.sync.dma_start(out=wt[:], in_=dram[:])
                wts[nm] = wt
            bq_t = const.tile([128, 1], F32, name="bq_t")
            nc.sync.dma_start(out=bq_t[:], in_=bq[:])
            bv_t = const.tile([128, 1], F32, name="bv_t")
            nc.sync.dma_start(out=bv_t[:], in_=bv[:])

            qt = const.tile([128, S], F16, name="qt")       # q^T in BOTH halves
            kt = const.tile([128, S], F16, name="kt")       # k^T: half (b%2) per block
            vaug = const.tile([128, NT, 80], F16, name="vaug")
            ptall = const.tile([128, NT, S], F16, name="ptall")  # exp(S^T) tiles
            oasb = const.tile([65, S], F16, name="oasb")
            ident = const.tile([128, 128], F16, name="ident")
            osb_all = const.tile([128, NT, H], F32, name="osb_all")

            make_identity(nc, ident[:])
            nc.vector.memset(vaug[:, :, 64], 1.0)

            # ---- input block DMAs (sync HWDGE ring, FIFO, prefetch depth 3)
            xqts, xkts, xvts = [], [], []

            def fetch_round(b):
                xkt = xkp.tile([128, EC, CB], F16, tag="xk", name=f"xkt{b}")
                nc.sync.dma_start(out=xkt[:], in_=xk[:, b])
                xkts.append(xkt)
                xvt = xvp.tile([128, EC, CB], F16, tag="xv", name=f"xvt{b}")
                nc.sync.dma_start(out=xvt[:], in_=xv[:, b])
                xvts.append(xvt)
                xqt = xqp.tile([128, EC, CB], F16, tag="xq", name=f"xqt{b}")
                nc.sync.dma_start(out=xqt[:], in_=xq[:, b])
                xqts.append(xqt)

            for b in range(5):
                fetch_round(b)

            work = pjp.tile([128, 2 * CB], F32, name="work")  # proj psum: A | B
            oa = oap.tile([65, S], F32, name="oa")            # AV accumulator

            # ---- PE warmup: keep TensorE busy through the DMA head so the
            # HAM clock gate reaches 2.4 GHz before real matmuls start.
            for w in range(32):
                nc.tensor.matmul(
                    work[0:128, 0:128], ident[:], ident[:],
                    start=True, stop=True, skip_group_check=True)

            # AV accumulation groups are PSUM-BANK granular: each oa bank
            # holds two 256-col columns; start on the bank's first MM only.
            av_bank_count = [0] * (NB // 2)
            av_pending = []

            def emit_cell(kb, qb):
                """Scores + exp for 256 keys (tiles 2kb, 2kb+1) x 256 q.
                The AV accumulation is queued and flushed a round later so
                the in-order PE FIFO never blocks on the v-evac ->
                DMA-transpose chain that produces vaug for fresh tiles."""
                g = (kb % 2) * 64
                cell = scp.tile([128, 2, CB], F32, tag="sc", name=f"sc{kb}_{qb}")
                for ti in range(2):
                    t = 2 * kb + ti
                    nc.tensor.matmul(
                        cell[:, ti, :],
                        kt[g:g + 64, t * 128:(t + 1) * 128],
                        qt[g:g + 64, qb * CB:(qb + 1) * CB],
                        start=True, stop=True, skip_group_check=True)
                nc.scalar.activation(
                    ptall[:, 2 * kb:2 * kb + 2, qb * CB:(qb + 1) * CB],
                    cell[:], Exp, scale=0.125)
                av_pending.append((kb, qb))

            def flush_av(cells):
                for kb, qb in cells:
                    bank = qb // 2
                    for ti in range(2):
                        t = 2 * kb + ti
                        cnt = av_bank_count[bank]
                        nc.tensor.matmul(
                            oa[:, qb * CB:(qb + 1) * CB],
                            vaug[:, t, 0:65],
                            ptall[:, t, qb * CB:(qb + 1) * CB],
                            start=(cnt == 0), stop=(cnt == 2 * NT - 1),
                            skip_group_check=True)
                        av_bank_count[bank] = cnt + 1

            for b in range(NB):
                bcols = slice(b * CB, (b + 1) * CB)
                if b + 5 < NB:
                    fetch_round(b + 5)

                # ---- pass B first: k_b in half (b%2), v_b in the other
                kh = (b % 2) * 64
                vh = 64 - kh
                for c in range(EC):
                    nc.tensor.matmul(
                        work[kh:kh + 64, CB:2 * CB], wts["k"][:, c, :],
                        xkts[b][:, c, :],
                        start=(c == 0), stop=(c == EC - 1),
                        skip_group_check=True)
                    nc.tensor.matmul(
                        work[vh:vh + 64, CB:2 * CB], wts["v"][:, c, :],
                        xvts[b][:, c, :],
                        start=(c == 0), stop=(c == EC - 1),
                        skip_group_check=True)
                vtb = vtp.tile([128, CB], F16, tag="vt", name=f"vtb{b}")
                nc.vector.tensor_scalar_add(
                    vtb[vh:vh + 64, :], work[vh:vh + 64, CB:2 * CB],
                    bv_t[vh:vh + 64])
                nc.sync.dma_start_transpose(
                    vaug[:, 2 * b:2 * b + 2, 0:64], vtb[vh:vh + 64, :])
                nc.vector.tensor_copy(
                    kt[kh:kh + 64, bcols], work[kh:kh + 64, CB:2 * CB])

                # row cells (kb=b, qb<b): scores vs already-built q columns
                for qb in range(b):
                    emit_cell(b, qb)

                # ---- pass A: q_b into both halves (col groups 0-63 | 64-127)
                for c in range(EC):
                    nc.tensor.matmul(
                        work[0:64, 0:CB], wts["q"][:, c, :], xqts[b][:, c, :],
                        start=(c == 0), stop=(c == EC - 1),
                        skip_group_check=True)
                    nc.tensor.matmul(
                        work[64:128, 0:CB], wts["q"][:, c, :], xqts[b][:, c, :],
                        start=(c == 0), stop=(c == EC - 1),
                        skip_group_check=True)
                nc.vector.tensor_scalar_add(qt[:, bcols], work[:, 0:CB], bq_t[:])

                # column cells (kb<=b, qb=b): parity alternates with kb
                for kb in range(b + 1):
                    emit_cell(kb, b)

                # AV for cells up to the previous round (vaug long since
                # landed) fills the PE while the next round's DMA streams.
                ready = [cq for cq in av_pending if cq[0] < b and cq[1] < b]
                av_pending[:] = [cq for cq in av_pending if cq not in ready]
                flush_av(ready)

            flush_av(av_pending)
            av_pending.clear()

            # ---- finalize: transpose, normalize, store ----
            out_r = out[:].rearrange("(t p) h -> p t h", p=128)
            for cq in range(4):
                nc.vector.tensor_copy(
                    oasb[:, cq * 512:(cq + 1) * 512],
                    oa[:, cq * 512:(cq + 1) * 512])
                for jj in range(4):
                    j = cq * 4 + jj
                    trt = scp.tile([128, 66], F16, tag="sc", name=f"tr{j}")
                    tr = trt[:, 0:65]
                    nc.tensor.transpose(
                        tr, oasb[:, j * 128:(j + 1) * 128], ident[0:65, 0:65])
                    rc = p5sb.tile([128, 1], F32, tag="rc", name=f"rc{j}")
                    nc.vector.reciprocal(rc[:], tr[:, 64:65])
                    nc.vector.tensor_scalar(
                        osb_all[:, j, :], tr[:, 0:64], rc[:], None,
                        op0=mybir.AluOpType.mult)
                nc.scalar.dma_start(
                    out=out_r[:, cq * 4:(cq + 1) * 4, :],
                    in_=osb_all[:, cq * 4:(cq + 1) * 4, :])

    nc.finalize()
    return nc


def get_nc():
    if "nc" not in _CACHE:
        _CACHE["nc"] = _build_nc()
    return _CACHE["nc"]


def _stage_x(x):
    # [S, E] f32 -> [128, NB, EC, CB] f16 with [p, b, c, s] = x[b*CB+s, c*128+p]
    xt = np.ascontiguousarray(x.T.astype(np.float16))          # [E, S]
    xt = xt.reshape(EC, 128, NB, CB).transpose(1, 2, 0, 3)     # [p, b, c, s]
    return np.ascontiguousarray(xt)


def make_in_maps(inputs):
    q = np.asarray(inputs["query"], np.float32)
    k = np.asarray(inputs["key_"], np.float32)
    v = np.asarray(inputs["value"], np.float32)
    wmats = {}
    for nm, key in (("wq", "Wq"), ("wk", "Wk"), ("wv", "Wv")):
        w = np.asarray(inputs[key], np.float32).astype(np.float16)  # [E, H]
        wmats[nm] = np.ascontiguousarray(
            w.reshape(EC, 128, H).transpose(1, 0, 2))               # [128, EC, H]
    bq = np.asarray(inputs["bq"], np.float32).reshape(H, 1)
    bv = np.asarray(inputs["bv"], np.float32).reshape(H, 1)
    bq_d = np.ascontiguousarray(np.tile(bq, (2, 1)))                # [128, 1]
    bv_d = np.ascontiguousarray(np.tile(bv, (2, 1)))
    in_maps = []
    for b in range(B):
        in_maps.append({
            "xq": _stage_x(q[b]),
            "xk": _stage_x(k[b]),
            "xv": _stage_x(v[b]),
            "wq": wmats["wq"], "wk": wmats["wk"], "wv": wmats["wv"],
            "bq": bq_d, "bv": bv_d,
        })
    return in_maps


def kernel(**inputs):
    nc = get_nc()
    in_maps = make_in_maps(inputs)
    res = run_bass_kernel_spmd(nc, in_maps, list(range(B)))
    return np.stack([res.results[b]["out"] for b in range(B)], axis=0)


# revision 28
# speedup vs baseline: 1.0423x; 1.0423x over previous
"""Trainium2 Bass kernel: single attention head (B=8, S=2048, E=1024, H=64).

Sharding: data-parallel over batch -- each of the 8 NeuronCores computes one
batch element's full attention. No collectives; every HBM byte read once.

v2 design (streaming wavefront):
  - Inputs are cast to fp16 HOST-side and staged as [128, blk, chunk, 256]
    so every 512KB block DMA is one contiguous 4KB line per partition on the
    fast HWDGE (sync) queue. Halves HBM traffic vs f32 and frees GpSimd.
  - Q/K/V stream in 8 interleaved column-block rounds (q_b, k_b, v_b). The
    softmax exp -- the ScalarE floor at ~27us -- starts at ~3us and runs
    continuously instead of waiting for all projections.
  - Projections run as column-tiled concurrent pairs (2x PE): pass A puts
    q_b in BOTH partition halves (array cols 0-63 // 64-127); pass B puts
    k_b in half (b%2) and v_b in the other. This materializes q and k in
    both SBUF partition halves with zero cross-partition copies.
  - Scores are row-tiled 2x: even-parity key tiles use array rows 0-63
    (kt/qt low half), odd tiles rows 64-127, concurrently, into different
    PSUM banks. Scores stay transposed (keys on partitions) so softmax
    rowsums ride a ones-column in the AV stationary.
  - exp on ScalarE (scale=1/8 fused) writes fp16 P tiles; AV accumulates
    [v|1]^T @ P into one [65, 2048] PSUM right behind each exp cell.
  - Finalize: PE transposes 128-col chunks (PSUM regions reuse the proj
    bank via f16 bitcast), VectorE reciprocal + scale, batched f32 DMA out.

PSUM: 1 bank proj (A/B slices) + 3 banks score cells + 4 banks AV = 8.
"""

import numpy as np

import concourse.bass as bass  # noqa: F401  (engine namespaces live on nc)
import concourse.mybir as mybir
import concourse.tile as tile
from concourse import bacc
from concourse.bass_utils import run_bass_kernel_spmd
from concourse.masks import make_identity

B, S, E, H = 8, 2048, 1024, 64
EC = E // 128    # contraction chunks (128 partitions each)
NB = 8           # column-block rounds
CB = S // NB     # 256 columns per block
NT = S // 128    # key tiles
F16 = mybir.dt.float16
F32 = mybir.dt.float32

_CACHE = {}


def _build_nc():
    nc = bacc.Bacc(None)
    xq = nc.declare_dram_parameter("xq", [128, NB, EC, CB], F16, isOutput=False)
    xk = nc.declare_dram_parameter("xk", [128, NB, EC, CB], F16, isOutput=False)
    xv = nc.declare_dram_parameter("xv", [128, NB, EC, CB], F16, isOutput=False)
    wq = nc.declare_dram_parameter("wq", [128, EC, H], F16, isOutput=False)
    wk = nc.declare_dram_parameter("wk", [128, EC, H], F16, isOutput=False)
    wv = nc.declare_dram_parameter("wv", [128, EC, H], F16, isOutput=False)
    bq = nc.declare_dram_parameter("bq", [128, 1], F32, isOutput=False)
    bv = nc.declare_dram_parameter("bv", [128, 1], F32, isOutput=False)
    out = nc.declare_dram_parameter("out", [S, H], F32, isOutput=True)

    Exp = mybir.ActivationFunctionType.Exp

    with tile.TileContext(nc) as tc:
        with tc.tile_pool(name="const", bufs=1) as const, \
             tc.tile_pool(name="xqp", bufs=6) as xqp, \
             tc.tile_pool(name="xkp", bufs=6) as xkp, \
             tc.tile_pool(name="xvp", bufs=6) as xvp, \
             tc.tile_pool(name="vtp", bufs=2) as vtp, \
             tc.tile_pool(name="p5sb", bufs=2) as p5sb, \
             tc.tile_pool(name="pjp", bufs=1, space="PSUM") as pjp, \
             tc.tile_pool(name="scp", bufs=3, space="PSUM") as scp, \
             tc.tile_pool(name="oap", bufs=1, space="PSUM") as oap:

            # ---- constants ----
            wts = {}
            for nm, dram in (("q", wq), ("k", wk), ("v", wv)):
                wt = const.tile([128, EC, H], F16, name=f"w{nm}")
                nc.sync.dma_start(out=wt[:], in_=dram[:])
                wts[nm] = wt
            bq_t = const.tile([128, 1], F32, name="bq_t")
            nc.sync.dma_start(out=bq_t[:], in_=bq[:])
            bv_t = const.tile([128, 1], F32, name="bv_t")
            nc.sync.dma_start(out=bv_t[:], in_=bv[:])

            qt = const.tile([128, S], F16, name="qt")       # q^T in BOTH halves
            kt = const.tile([128, S], F16, name="kt")       # k^T: half (b%2) per block
            vaug = const.tile([128, NT, 80], F16, name="vaug")
            ptall = const.tile([128, NT, S], F16, name="ptall")  # exp(S^T) tiles
            oasb = const.tile([65, S], F16, name="oasb")
            ident = const.tile([128, 128], F16, name="ident")
            osb_all = const.tile([128, NT, H], F32, name="osb_all")

            make_identity(nc, ident[:])
            nc.vector.memset(vaug[:, :, 64], 1.0)

            # ---- input block DMAs (sync HWDGE ring, FIFO, prefetch depth 3)
            xqts, xkts, xvts = [], [], []

            def fetch_round(b):
                xkt = xkp.tile([128, EC, CB], F16, tag="xk", name=f"xkt{b}")
                nc.sync.dma_start(out=xkt[:], in_=xk[:, b])
                xkts.append(xkt)
                xvt = xvp.tile([128, EC, CB], F16, tag="xv", name=f"xvt{b}")
                nc.sync.dma_start(out=xvt[:], in_=xv[:, b])
                xvts.append(xvt)
                xqt = xqp.tile([128, EC, CB], F16, tag="xq", name=f"xqt{b}")
                nc.sync.dma_start(out=xqt[:], in_=xq[:, b])
                xqts.append(xqt)

            for b in range(5):
                fetch_round(b)

            work = pjp.tile([128, 2 * CB], F32, name="work")  # proj psum: A | B
            oa = oap.tile([65, S], F32, name="oa")            # AV accumulator

            # ---- PE warmup: keep TensorE busy through the DMA head so the
            # HAM clock gate reaches 2.4 GHz before real matmuls start.
            for w in range(32):
                nc.tensor.matmul(
                    work[0:128, 0:128], ident[:], ident[:],
                    start=True, stop=True, skip_group_check=True)

            # AV accumulation groups are PSUM-BANK granular: each oa bank
            # holds two 256-col columns; start on the bank's first MM only.
            av_bank_count = [0] * (NB // 2)
            av_pending = []

            def emit_cell(kb, qb):
                """Scores + exp for 256 keys (tiles 2kb, 2kb+1) x 256 q.
                The AV accumulation is queued and flushed a round later so
                the in-order PE FIFO never blocks on the v-evac ->
                DMA-transpose chain that produces vaug for fresh tiles."""
                g = (kb % 2) * 64
                cell = scp.tile([128, 2, CB], F32, tag="sc", name=f"sc{kb}_{qb}")
                for ti in range(2):
                    t = 2 * kb + ti
                    nc.tensor.matmul(
                        cell[:, ti, :],
                        kt[g:g + 64, t * 128:(t + 1) * 128],
                        qt[g:g + 64, qb * CB:(qb + 1) * CB],
                        start=True, stop=True, skip_group_check=True)
                nc.scalar.activation(
                    ptall[:, 2 * kb:2 * kb + 2, qb * CB:(qb + 1) * CB],
                    cell[:], Exp, scale=0.125)
                av_pending.append((kb, qb))

            av_ready = []

            def emit_av(n):
                """Emit up to n queued AV matmuls (always dependency-ready) —
                filler work that keeps the in-order PE FIFO dense."""
                while n > 0 and av_ready:
                    kb, qb = av_ready.pop(0)
                    bank = qb // 2
                    for ti in range(2):
                        t = 2 * kb + ti
                        cnt = av_bank_count[bank]
                        nc.tensor.matmul(
                            oa[:, qb * CB:(qb + 1) * CB],
                            vaug[:, t, 0:65],
                            ptall[:, t, qb * CB:(qb + 1) * CB],
                            start=(cnt == 0), stop=(cnt == 2 * NT - 1),
                            skip_group_check=True)
                        av_bank_count[bank] = cnt + 1
                    n -= 1

            for b in range(NB):
                bcols = slice(b * CB, (b + 1) * CB)
                if b + 5 < NB:
                    fetch_round(b + 5)

                # cells from rounds <= b-1 are AV-ready now (their vaug and
                # exp landed a round ago)
                av_ready.extend(av_pending)
                av_pending.clear()

                emit_av(3)  # fill the PE while round b's blocks land

                # ---- pass B first: k_b in half (b%2), v_b in the other
                kh = (b % 2) * 64
                vh = 64 - kh
                for c in range(EC):
                    nc.tensor.matmul(
                        work[kh:kh + 64, CB:2 * CB], wts["k"][:, c, :],
                        xkts[b][:, c, :],
                        start=(c == 0), stop=(c == EC - 1),
                        skip_group_check=True)
                    nc.tensor.matmul(
                        work[vh:vh + 64, CB:2 * CB], wts["v"][:, c, :],
                        xvts[b][:, c, :],
                        start=(c == 0), stop=(c == EC - 1),
                        skip_group_check=True)
                vtb = vtp.tile([128, CB], F16, tag="vt", name=f"vtb{b}")
                nc.vector.tensor_scalar_add(
                    vtb[vh:vh + 64, :], work[vh:vh + 64, CB:2 * CB],
                    bv_t[vh:vh + 64])
                nc.sync.dma_start_transpose(
                    vaug[:, 2 * b:2 * b + 2, 0:64], vtb[vh:vh + 64, :])
                nc.vector.tensor_copy(
                    kt[kh:kh + 64, bcols], work[kh:kh + 64, CB:2 * CB])

                # ---- pass A right behind (no evac dependency): q_b dual
                for c in range(EC):
                    nc.tensor.matmul(
                        work[0:64, 0:CB], wts["q"][:, c, :], xqts[b][:, c, :],
                        start=(c == 0), stop=(c == EC - 1),
                        skip_group_check=True)
                    nc.tensor.matmul(
                        work[64:128, 0:CB], wts["q"][:, c, :], xqts[b][:, c, :],
                        start=(c == 0), stop=(c == EC - 1),
                        skip_group_check=True)
                nc.vector.tensor_scalar_add(qt[:, bcols], work[:, 0:CB], bq_t[:])

                # row cells (kb=b, qb<b) then column cells (kb<=b, qb=b),
                # AV filler between cells to absorb exp-slot waits
                for qb in range(b):
                    emit_cell(b, qb)
                    emit_av(2)
                for kb in range(b + 1):
                    emit_cell(kb, b)
                    emit_av(2)

            av_ready.extend(av_pending)
            av_pending.clear()
            emit_av(len(av_ready))

            # ---- finalize: transpose, normalize, store ----
            out_r = out[:].rearrange("(t p) h -> p t h", p=128)
            for cq in range(4):
                nc.vector.tensor_copy(
                    oasb[:, cq * 512:(cq + 1) * 512],
                    oa[:, cq * 512:(cq + 1) * 512])
                for jj in range(4):
                    j = cq * 4 + jj
                    trt = scp.tile([128, 66], F16, tag="sc", name=f"tr{j}")
                    tr = trt[:, 0:65]
                    nc.tensor.transpose(
                        tr, oasb[:, j * 128:(j + 1) * 128], ident[0:65, 0:65])
                    rc = p5sb.tile([128, 1], F32, tag="rc", name=f"rc{j}")
                    nc.vector.reciprocal(rc[:], tr[:, 64:65])
                    nc.vector.tensor_scalar(
                        osb_all[:, j, :], tr[:, 0:64], rc[:], None,
                        op0=mybir.AluOpType.mult)
                nc.scalar.dma_start(
                    out=out_r[:, cq * 4:(cq + 1) * 4, :],
                    in_=osb_all[:, cq * 4:(cq + 1) * 4, :])

    nc.finalize()
    return nc


def get_nc():
    if "nc" not in _CACHE:
        _CACHE["nc"] = _build_nc()
    return _CACHE["nc"]


def _stage_x(x):
    # [S, E] f32 -> [128, NB, EC, CB] f16 with [p, b, c, s] = x[b*CB+s, c*128+p]
    xt = np.ascontiguousarray(x.T.astype(np.float16))          # [E, S]
    xt = xt.reshape(EC, 128, NB, CB).transpose(1, 2, 0, 3)     # [p, b, c, s]
    return np.ascontiguousarray(xt)


def make_in_maps(inputs):
    q = np.asarray(inputs["query"], np.float32)
    k = np.asarray(inputs["key_"], np.float32)
    v = np.asarray(inputs["value"], np.float32)
    wmats = {}
    for nm, key in (("wq", "Wq"), ("wk", "Wk"), ("wv", "Wv")):
        w = np.asarray(inputs[key], np.float32).astype(np.float16)  # [E, H]
        wmats[nm] = np.ascontiguousarray(
            w.reshape(EC, 128, H).transpose(1, 0, 2))               # [128, EC, H]
    bq = np.asarray(inputs["bq"], np.float32).reshape(H, 1)
    bv = np.asarray(inputs["bv"], np.float32).reshape(H, 1)
    bq_d = np.ascontiguousarray(np.tile(bq, (2, 1)))                # [128, 1]
    bv_d = np.ascontiguousarray(np.tile(bv, (2, 1)))
    in_maps = []
    for b in range(B):
        in_maps.append({
            "xq": _stage_x(q[b]),
            "xk": _stage_x(k[b]),
            "xv": _stage_x(v[b]),
            "wq": wmats["wq"], "wk": wmats["wk"], "wv": wmats["wv"],
            "bq": bq_d, "bv": bv_d,
        })
    return in_maps


def kernel(**inputs):
    nc = get_nc()
    in_maps = make_in_maps(inputs)
    res = run_bass_kernel_spmd(nc, in_maps, list(range(B)))
    return np.stack([res.results[b]["out"] for b in range(B)], axis=0)


# revision 33
# speedup vs baseline: 1.0617x; 1.0187x over previous
"""Trainium2 Bass kernel: single attention head (B=8, S=2048, E=1024, H=64).

Sharding: data-parallel over batch -- each of the 8 NeuronCores computes one
batch element's full attention. No collectives; every HBM byte read once.

v2 design (streaming wavefront):
  - Inputs are cast to fp16 HOST-side and staged as [128, blk, chunk, 256]
    so every 512KB block DMA is one contiguous 4KB line per partition on the
    fast HWDGE (sync) queue. Halves HBM traffic vs f32 and frees GpSimd.
  - Q/K/V stream in 8 interleaved column-block rounds (q_b, k_b, v_b). The
    softmax exp -- the ScalarE floor at ~27us -- starts at ~3us and runs
    continuously instead of waiting for all projections.
  - Projections run as column-tiled concurrent pairs (2x PE): pass A puts
    q_b in BOTH partition halves (array cols 0-63 // 64-127); pass B puts
    k_b in half (b%2) and v_b in the other. This materializes q and k in
    both SBUF partition halves with zero cross-partition copies.
  - Scores are row-tiled 2x: even-parity key tiles use array rows 0-63
    (kt/qt low half), odd tiles rows 64-127, concurrently, into different
    PSUM banks. Scores stay transposed (keys on partitions) so softmax
    rowsums ride a ones-column in the AV stationary.
  - exp on ScalarE (scale=1/8 fused) writes fp16 P tiles; AV accumulates
    [v|1]^T @ P into one [65, 2048] PSUM right behind each exp cell.
  - Finalize: PE transposes 128-col chunks (PSUM regions reuse the proj
    bank via f16 bitcast), VectorE reciprocal + scale, batched f32 DMA out.

PSUM: 1 bank proj (A/B slices) + 3 banks score cells + 4 banks AV = 8.
"""

import numpy as np

import concourse.bass as bass  # noqa: F401  (engine namespaces live on nc)
import concourse.mybir as mybir
import concourse.tile as tile
from concourse import bacc
from concourse.bass_utils import run_bass_kernel_spmd
from concourse.masks import make_identity

B, S, E, H = 8, 2048, 1024, 64
EC = E // 128    # contraction chunks (128 partitions each)
NB = 8           # column-block rounds
CB = S // NB     # 256 columns per block
NT = S // 128    # key tiles
F16 = mybir.dt.float16
F32 = mybir.dt.float32

_CACHE = {}


def _build_nc():
    nc = bacc.Bacc(None)
    xq = nc.declare_dram_parameter("xq", [128, NB, EC, CB], F16, isOutput=False)
    xk = nc.declare_dram_parameter("xk", [128, NB, EC, CB], F16, isOutput=False)
    xv = nc.declare_dram_parameter("xv", [128, NB, EC, CB], F16, isOutput=False)
    wq = nc.declare_dram_parameter("wq", [128, EC, H], F16, isOutput=False)
    wk = nc.declare_dram_parameter("wk", [128, EC, H], F16, isOutput=False)
    wv = nc.declare_dram_parameter("wv", [128, EC, H], F16, isOutput=False)
    bq = nc.declare_dram_parameter("bq", [128, 1], F32, isOutput=False)
    bv = nc.declare_dram_parameter("bv", [128, 1], F32, isOutput=False)
    out = nc.declare_dram_parameter("out", [S, H], F32, isOutput=True)

    Exp = mybir.ActivationFunctionType.Exp

    with tile.TileContext(nc) as tc:
        with tc.tile_pool(name="const", bufs=1) as const, \
             tc.tile_pool(name="xqp", bufs=6) as xqp, \
             tc.tile_pool(name="xkp", bufs=6) as xkp, \
             tc.tile_pool(name="xvp", bufs=6) as xvp, \
             tc.tile_pool(name="vtp", bufs=2) as vtp, \
             tc.tile_pool(name="p5sb", bufs=2) as p5sb, \
             tc.tile_pool(name="pjp", bufs=1, space="PSUM") as pjp, \
             tc.tile_pool(name="scp", bufs=3, space="PSUM") as scp, \
             tc.tile_pool(name="oap", bufs=1, space="PSUM") as oap:

            # ---- constants ----
            wts = {}
            for nm, dram in (("q", wq), ("k", wk), ("v", wv)):
                wt = const.tile([128, EC, H], F16, name=f"w{nm}")
                nc.sync.dma_start(out=wt[:], in_=dram[:])
                wts[nm] = wt
            bq_t = const.tile([128, 1], F32, name="bq_t")
            nc.sync.dma_start(out=bq_t[:], in_=bq[:])
            bv_t = const.tile([128, 1], F32, name="bv_t")
            nc.sync.dma_start(out=bv_t[:], in_=bv[:])

            qt = const.tile([128, S], F16, name="qt")       # q^T in BOTH halves
            kt = const.tile([128, S], F16, name="kt")       # k^T: half (b%2) per block
            vaug = const.tile([128, NT, 80], F16, name="vaug")
            ptall = const.tile([128, NT, S], F16, name="ptall")  # exp(S^T) tiles
            oasb = const.tile([65, S], F16, name="oasb")
            ident = const.tile([128, 128], F16, name="ident")
            osb_all = const.tile([128, NT, H], F32, name="osb_all")

            make_identity(nc, ident[:])
            nc.vector.memset(vaug[:, :, 64], 1.0)

            # ---- input block DMAs (sync HWDGE ring, FIFO, prefetch depth 3)
            xqts, xkts, xvts = [], [], []

            def fetch_round(b):
                xkt = xkp.tile([128, EC, CB], F16, tag="xk", name=f"xkt{b}")
                nc.sync.dma_start(out=xkt[:], in_=xk[:, b])
                xkts.append(xkt)
                xvt = xvp.tile([128, EC, CB], F16, tag="xv", name=f"xvt{b}")
                nc.sync.dma_start(out=xvt[:], in_=xv[:, b])
                xvts.append(xvt)
                xqt = xqp.tile([128, EC, CB], F16, tag="xq", name=f"xqt{b}")
                nc.sync.dma_start(out=xqt[:], in_=xq[:, b])
                xqts.append(xqt)

            for b in range(5):
                fetch_round(b)

            work = pjp.tile([128, 2 * CB], F32, name="work")  # proj psum: A | B
            oa = oap.tile([65, S], F32, name="oa")            # AV accumulator

            # ---- PE warmup: keep TensorE busy through the DMA head so the
            # HAM clock gate reaches 2.4 GHz before real matmuls start.
            for w in range(32):
                nc.tensor.matmul(
                    work[0:128, 0:128], ident[:], ident[:],
                    start=True, stop=True, skip_group_check=True)

            # AV accumulation groups are PSUM-BANK granular: each oa bank
            # holds two 256-col columns; start on the bank's first MM only.
            av_bank_count = [0] * (NB // 2)
            av_pending = []

            def emit_cell(kb, qb):
                """Scores + exp for 256 keys (tiles 2kb, 2kb+1) x 256 q.
                The AV accumulation is queued and flushed a round later so
                the in-order PE FIFO never blocks on the v-evac ->
                DMA-transpose chain that produces vaug for fresh tiles."""
                g = (kb % 2) * 64
                cell = scp.tile([128, 2, CB], F32, tag="sc", name=f"sc{kb}_{qb}")
                for ti in range(2):
                    t = 2 * kb + ti
                    nc.tensor.matmul(
                        cell[:, ti, :],
                        kt[g:g + 64, t * 128:(t + 1) * 128],
                        qt[g:g + 64, qb * CB:(qb + 1) * CB],
                        start=True, stop=True, skip_group_check=True)
                nc.scalar.activation(
                    ptall[:, 2 * kb:2 * kb + 2, qb * CB:(qb + 1) * CB],
                    cell[:], Exp, scale=0.125)
                av_pending.append((kb, qb))

            av_ready = []

            def emit_dummy(n):
                """Dependency-free warm-keeper matmuls into the proj bank
                (lazy PSUM zeroing leaves prior values readable)."""
                for _ in range(n):
                    nc.tensor.matmul(
                        work[0:128, 0:128], ident[:], ident[:],
                        start=True, stop=True, skip_group_check=True)

            def emit_av(n, pad=False):
                """Emit up to n queued AV matmuls (always dependency-ready) —
                filler work that keeps the in-order PE FIFO dense. With
                pad=True, tops up with dummy matmuls when the AV queue runs
                dry (early rounds) so the HAM clock gate stays warm."""
                while n > 0 and av_ready:
                    kb, qb = av_ready.pop(0)
                    bank = qb // 2
                    for ti in range(2):
                        t = 2 * kb + ti
                        cnt = av_bank_count[bank]
                        nc.tensor.matmul(
                            oa[:, qb * CB:(qb + 1) * CB],
                            vaug[:, t, 0:65],
                            ptall[:, t, qb * CB:(qb + 1) * CB],
                            start=(cnt == 0), stop=(cnt == 2 * NT - 1),
                            skip_group_check=True)
                        av_bank_count[bank] = cnt + 1
                    n -= 1
                if pad and n > 0:
                    emit_dummy(2 * n)

            for b in range(NB):
                bcols = slice(b * CB, (b + 1) * CB)
                if b + 5 < NB:
                    fetch_round(b + 5)

                # cells from rounds <= b-1 are AV-ready now (their vaug and
                # exp landed a round ago)
                av_ready.extend(av_pending)
                av_pending.clear()

                emit_av(3, pad=(b < 4))  # fill the PE while blocks land

                # ---- pass B first: k_b in half (b%2), v_b in the other
                kh = (b % 2) * 64
                vh = 64 - kh
                for c in range(EC):
                    nc.tensor.matmul(
                        work[kh:kh + 64, CB:2 * CB], wts["k"][:, c, :],
                        xkts[b][:, c, :],
                        start=(c == 0), stop=(c == EC - 1),
                        skip_group_check=True)
                    nc.tensor.matmul(
                        work[vh:vh + 64, CB:2 * CB], wts["v"][:, c, :],
                        xvts[b][:, c, :],
                        start=(c == 0), stop=(c == EC - 1),
                        skip_group_check=True)
                vtb = vtp.tile([128, CB], F16, tag="vt", name=f"vtb{b}")
                nc.vector.tensor_scalar_add(
                    vtb[vh:vh + 64, :], work[vh:vh + 64, CB:2 * CB],
                    bv_t[vh:vh + 64])
                nc.sync.dma_start_transpose(
                    vaug[:, 2 * b:2 * b + 2, 0:64], vtb[vh:vh + 64, :])
                nc.vector.tensor_copy(
                    kt[kh:kh + 64, bcols], work[kh:kh + 64, CB:2 * CB])

                # ---- pass A right behind (no evac dependency): q_b dual
                for c in range(EC):
                    nc.tensor.matmul(
                        work[0:64, 0:CB], wts["q"][:, c, :], xqts[b][:, c, :],
                        start=(c == 0), stop=(c == EC - 1),
                        skip_group_check=True)
                    nc.tensor.matmul(
                        work[64:128, 0:CB], wts["q"][:, c, :], xqts[b][:, c, :],
                        start=(c == 0), stop=(c == EC - 1),
                        skip_group_check=True)
                nc.vector.tensor_scalar_add(qt[:, bcols], work[:, 0:CB], bq_t[:])

                # row cells (kb=b, qb<b) then column cells (kb<=b, qb=b),
                # AV filler between cells to absorb exp-slot waits
                for qb in range(b):
                    emit_cell(b, qb)
                    emit_av(2, pad=(b < 4))
                for kb in range(b + 1):
                    emit_cell(kb, b)
                    emit_av(2, pad=(b < 4))

            av_ready.extend(av_pending)
            av_pending.clear()
            emit_av(len(av_ready))

            # ---- finalize: transpose, normalize, store ----
            out_r = out[:].rearrange("(t p) h -> p t h", p=128)
            for cq in range(4):
                nc.vector.tensor_copy(
                    oasb[:, cq * 512:(cq + 1) * 512],
                    oa[:, cq * 512:(cq + 1) * 512])
                for jj in range(4):
                    j = cq * 4 + jj
                    trt = scp.tile([128, 66], F16, tag="sc", name=f"tr{j}")
                    tr = trt[:, 0:65]
                    nc.tensor.transpose(
                        tr, oasb[:, j * 128:(j + 1) * 128], ident[0:65, 0:65])
                    rc = p5sb.tile([128, 1], F32, tag="rc", name=f"rc{j}")
                    nc.vector.reciprocal(rc[:], tr[:, 64:65])
                    nc.vector.tensor_scalar(
                        osb_all[:, j, :], tr[:, 0:64], rc[:], None,
                        op0=mybir.AluOpType.mult)
                nc.scalar.dma_start(
                    out=out_r[:, cq * 4:(cq + 1) * 4, :],
                    in_=osb_all[:, cq * 4:(cq + 1) * 4, :])

    nc.finalize()
    return nc


def get_nc():
    if "nc" not in _CACHE:
        _CACHE["nc"] = _build_nc()
    return _CACHE["nc"]


def _stage_x(x):
    # [S, E] f32 -> [128, NB, EC, CB] f16 with [p, b, c, s] = x[b*CB+s, c*128+p]
    xt = np.ascontiguousarray(x.T.astype(np.float16))          # [E, S]
    xt = xt.reshape(EC, 128, NB, CB).transpose(1, 2, 0, 3)     # [p, b, c, s]
    return np.ascontiguousarray(xt)


def make_in_maps(inputs):
    q = np.asarray(inputs["query"], np.float32)
    k = np.asarray(inputs["key_"], np.float32)
    v = np.asarray(inputs["value"], np.float32)
    wmats = {}
    for nm, key in (("wq", "Wq"), ("wk", "Wk"), ("wv", "Wv")):
        w = np.asarray(inputs[key], np.float32).astype(np.float16)  # [E, H]
        wmats[nm] = np.ascontiguousarray(
            w.reshape(EC, 128, H).transpose(1, 0, 2))               # [128, EC, H]
    bq = np.asarray(inputs["bq"], np.float32).reshape(H, 1)
    bv = np.asarray(inputs["bv"], np.float32).reshape(H, 1)
    bq_d = np.ascontiguousarray(np.tile(bq, (2, 1)))                # [128, 1]
    bv_d = np.ascontiguousarray(np.tile(bv, (2, 1)))
    in_maps = []
    for b in range(B):
        in_maps.append({
            "xq": _stage_x(q[b]),
            "xk": _stage_x(k[b]),
            "xv": _stage_x(v[b]),
            "wq": wmats["wq"], "wk": wmats["wk"], "wv": wmats["wv"],
            "bq": bq_d, "bv": bv_d,
        })
    return in_maps


def kernel(**inputs):
    nc = get_nc()
    in_maps = make_in_maps(inputs)
    res = run_bass_kernel_spmd(nc, in_maps, list(range(B)))
    return np.stack([res.results[b]["out"] for b in range(B)], axis=0)


# revision 39
# speedup vs baseline: 1.1204x; 1.0553x over previous
"""Trainium2 Bass kernel: single attention head (B=8, S=2048, E=1024, H=64).

Sharding: data-parallel over batch -- each of the 8 NeuronCores computes one
batch element's full attention. No collectives; every HBM byte read once.

v8 design (q-first, k-streaming, big exp slabs):
  - Inputs cast to fp16 HOST-side, staged [128, blk, chunk, 256] so every
    512KB block DMA is one contiguous 4KB line per partition on the HWDGE
    (sync) queue. Half the HBM traffic of the f32 original.
  - Phase Q (~0-12us): all 8 q-blocks stream in; each is projected into
    BOTH SBUF partition halves via column-tiled concurrent matmul pairs
    (array cols 0-63 / 64-127), enabling row-tiled scores later. Dummy
    warm-keeper matmuls pad the gaps so the PE HAM clock gate reaches and
    holds 2.4 GHz.
  - Phase KV: k/v block-pairs stream; per block a col-tiled k||v projection
    pass, then per key tile: row-tiled scores at N=512 (tiles from even
    blocks run in array rows 0-63 concurrently with odd-block tiles in rows
    64-127, different PSUM banks), exp on ScalarE over [128, 1024] slabs
    (the ScalarE stream is the global bottleneck: ~36us), and AV matmuls
    ([v|1]^T @ P into a [65, 2048] accumulator) used as always-ready FILLER
    between score groups so the in-order PE FIFO never idles.
  - Scores stay transposed (keys on partitions); softmax rowsums ride the
    ones column of the AV stationary; bk cancels in softmax; bq/bv fold
    into the projection evacuations.
  - Finalize: PE transposes 128-col chunks, VectorE reciprocal + scale,
    batched f32 DMA out.

PSUM: 4 x 2-bank rotating slots (projections + score slabs + finalize
transposes) + 4 banks AV accumulator = 8 banks exactly.
"""

import numpy as np

import concourse.bass as bass  # noqa: F401  (engine namespaces live on nc)
import concourse.mybir as mybir
import concourse.tile as tile
from concourse import bacc
from concourse.bass_utils import run_bass_kernel_spmd
from concourse.masks import make_identity

B, S, E, H = 8, 2048, 1024, 64
EC = E // 128    # contraction chunks (128 partitions each)
NB = 8           # column blocks per tensor
CB = S // NB     # 256 columns per block
NT = S // 128    # key tiles
F16 = mybir.dt.float16
F32 = mybir.dt.float32

_CACHE = {}


def _build_nc():
    nc = bacc.Bacc(None)
    xq = nc.declare_dram_parameter("xq", [128, NB, EC, CB], F16, isOutput=False)
    xk = nc.declare_dram_parameter("xk", [128, NB, EC, CB], F16, isOutput=False)
    xv = nc.declare_dram_parameter("xv", [128, NB, EC, CB], F16, isOutput=False)
    wq = nc.declare_dram_parameter("wq", [128, EC, H], F16, isOutput=False)
    wk = nc.declare_dram_parameter("wk", [128, EC, H], F16, isOutput=False)
    wv = nc.declare_dram_parameter("wv", [128, EC, H], F16, isOutput=False)
    bq = nc.declare_dram_parameter("bq", [128, 1], F32, isOutput=False)
    bv = nc.declare_dram_parameter("bv", [128, 1], F32, isOutput=False)
    out = nc.declare_dram_parameter("out", [S, H], F32, isOutput=True)

    Exp = mybir.ActivationFunctionType.Exp

    with tile.TileContext(nc) as tc:
        with tc.tile_pool(name="const", bufs=1) as const, \
             tc.tile_pool(name="xqp", bufs=3) as xqp, \
             tc.tile_pool(name="xkp", bufs=4) as xkp, \
             tc.tile_pool(name="xvp", bufs=4) as xvp, \
             tc.tile_pool(name="vtp", bufs=2) as vtp, \
             tc.tile_pool(name="p5sb", bufs=2) as p5sb, \
             tc.tile_pool(name="psp", bufs=2, space="PSUM") as psp, \
             tc.tile_pool(name="oap", bufs=1, space="PSUM") as oap:

            # ---- constants ----
            wts = {}
            for nm, dram in (("q", wq), ("k", wk), ("v", wv)):
                wt = const.tile([128, EC, H], F16, name=f"w{nm}")
                nc.sync.dma_start(out=wt[:], in_=dram[:])
                wts[nm] = wt
            bq_t = const.tile([128, 1], F32, name="bq_t")
            nc.sync.dma_start(out=bq_t[:], in_=bq[:])
            bv_t = const.tile([128, 1], F32, name="bv_t")
            nc.sync.dma_start(out=bv_t[:], in_=bv[:])

            qt = const.tile([128, S], F16, name="qt")       # q^T in BOTH halves
            kt = const.tile([128, S], F16, name="kt")       # k^T: half (b%2)
            vaug = const.tile([128, NT, 80], F16, name="vaug")
            ptall = const.tile([128, NT, S], F16, name="ptall")
            oasb = const.tile([65, S], F16, name="oasb")
            ident = const.tile([128, 128], F16, name="ident")
            osb_all = const.tile([128, NT, H], F32, name="osb_all")

            make_identity(nc, ident[:])
            nc.vector.memset(vaug[:, :, 64], 1.0)

            oa = oap.tile([65, S], F32, name="oa")          # AV accumulator

            def slot(name):
                return psp.tile([128, 1024], F32, tag="ps", name=name)

            # ---- input DMAs (sync HWDGE FIFO): all q first, then k/v pairs
            xqts = [xqp.tile([128, EC, CB], F16, tag="xq", name=f"xqt{b}")
                    for b in range(NB)]
            for b in range(NB):
                nc.sync.dma_start(out=xqts[b][:], in_=xq[:, b])
            xkts, xvts = [], []

            def fetch_kv(b):
                xkt = xkp.tile([128, EC, CB], F16, tag="xk", name=f"xkt{b}")
                nc.sync.dma_start(out=xkt[:], in_=xk[:, b])
                xkts.append(xkt)
                xvt = xvp.tile([128, EC, CB], F16, tag="xv", name=f"xvt{b}")
                nc.sync.dma_start(out=xvt[:], in_=xv[:, b])
                xvts.append(xvt)

            for b in range(4):
                fetch_kv(b)

            # ---- PE warm-keeper ----
            wslot = slot("warm")

            def emit_dummy(n):
                for _ in range(n):
                    nc.tensor.matmul(
                        wslot[0:128, 0:128], ident[:], ident[:],
                        start=True, stop=True, skip_group_check=True)

            emit_dummy(30)

            # ---- AV queue: always-ready filler matmuls ----
            # entries (t, qh): [v_t | 1]^T @ P_t over q columns qh*1024:+1024
            av_ready = []
            av_bank_count = [0] * 4

            def emit_av(n):
                while n > 0 and av_ready:
                    t, qh = av_ready.pop(0)
                    for sg in range(2):
                        seg = 2 * qh + sg
                        cnt = av_bank_count[seg]
                        nc.tensor.matmul(
                            oa[:, seg * 512:(seg + 1) * 512],
                            vaug[:, t, 0:65],
                            ptall[:, t, seg * 512:(seg + 1) * 512],
                            start=(cnt == 0), stop=(cnt == NT - 1),
                            skip_group_check=True)
                        av_bank_count[seg] = cnt + 1
                    n -= 1

            # ---- phase Q: project all q blocks into both halves ----
            for qb in range(NB):
                ps = slot(f"pq{qb}")
                for c in range(EC):
                    nc.tensor.matmul(
                        ps[0:64, 0:CB], wts["q"][:, c, :], xqts[qb][:, c, :],
                        start=(c == 0), stop=(c == EC - 1),
                        skip_group_check=True)
                    nc.tensor.matmul(
                        ps[64:128, 0:CB], wts["q"][:, c, :], xqts[qb][:, c, :],
                        start=(c == 0), stop=(c == EC - 1),
                        skip_group_check=True)
                nc.vector.tensor_scalar_add(
                    qt[:, qb * CB:(qb + 1) * CB], ps[:, 0:CB], bq_t[:])

            # ---- phase KV: stream k/v block pairs ----
            for j in range(4):                 # block pair (2j, 2j+1)
                for sidx in range(2):
                    b = 2 * j + sidx
                    if b + 4 < NB:
                        fetch_kv(b + 4)
                    kh = (b % 2) * 64
                    vh = 64 - kh
                    ps = slot(f"pkv{b}")
                    for c in range(EC):
                        nc.tensor.matmul(
                            ps[kh:kh + 64, 0:CB], wts["k"][:, c, :],
                            xkts[b][:, c, :],
                            start=(c == 0), stop=(c == EC - 1),
                            skip_group_check=True)
                        nc.tensor.matmul(
                            ps[vh:vh + 64, 0:CB], wts["v"][:, c, :],
                            xvts[b][:, c, :],
                            start=(c == 0), stop=(c == EC - 1),
                            skip_group_check=True)
                    vtb = vtp.tile([128, CB], F16, tag="vt", name=f"vtb{b}")
                    nc.vector.tensor_scalar_add(
                        vtb[vh:vh + 64, :], ps[vh:vh + 64, 0:CB],
                        bv_t[vh:vh + 64])
                    nc.sync.dma_start_transpose(
                        vaug[:, 2 * b:2 * b + 2, 0:64], vtb[vh:vh + 64, :])
                    nc.vector.tensor_copy(
                        kt[kh:kh + 64, b * CB:(b + 1) * CB],
                        ps[kh:kh + 64, 0:CB])
                    emit_av(2)

                # scores + exp for tiles {4j, 4j+1} (rows 0-63) paired with
                # {4j+2, 4j+3} (rows 64-127); two 1024-col slabs per tile
                for qh in range(2):
                    qs = slice(qh * 1024, (qh + 1) * 1024)
                    for pi in range(2):
                        ta = 4 * j + pi        # block 2j   -> half (2j)%2 = 0
                        tb_ = 4 * j + 2 + pi   # block 2j+1 -> half 1
                        ga = ((ta // 2) % 2) * 64
                        gb = ((tb_ // 2) % 2) * 64
                        sa = slot(f"sa{ta}_{qh}")
                        sb_ = slot(f"sb{tb_}_{qh}")
                        for seg in range(2):
                            cs = slice(qh * 1024 + seg * 512,
                                       qh * 1024 + (seg + 1) * 512)
                            nc.tensor.matmul(
                                sa[:, seg * 512:(seg + 1) * 512],
                                kt[ga:ga + 64, ta * 128:(ta + 1) * 128],
                                qt[ga:ga + 64, cs],
                                start=True, stop=True, skip_group_check=True)
                            nc.tensor.matmul(
                                sb_[:, seg * 512:(seg + 1) * 512],
                                kt[gb:gb + 64, tb_ * 128:(tb_ + 1) * 128],
                                qt[gb:gb + 64, cs],
                                start=True, stop=True, skip_group_check=True)
                        nc.scalar.activation(
                            ptall[:, ta, qs], sa[:], Exp, scale=0.125)
                        nc.scalar.activation(
                            ptall[:, tb_, qs], sb_[:], Exp, scale=0.125)
                        emit_av(2)
                        av_ready.append((ta, qh))
                        av_ready.append((tb_, qh))

            emit_av(len(av_ready))

            # ---- finalize: transpose, normalize, store ----
            out_r = out[:].rearrange("(t p) h -> p t h", p=128)
            for cq in range(4):
                nc.vector.tensor_copy(
                    oasb[:, cq * 512:(cq + 1) * 512],
                    oa[:, cq * 512:(cq + 1) * 512])
                for jj in range(4):
                    j = cq * 4 + jj
                    trt = psp.tile([128, 66], F16, tag="ps", name=f"tr{j}")
                    tr = trt[:, 0:65]
                    nc.tensor.transpose(
                        tr, oasb[:, j * 128:(j + 1) * 128], ident[0:65, 0:65])
                    rc = p5sb.tile([128, 1], F32, tag="rc", name=f"rc{j}")
                    nc.vector.reciprocal(rc[:], tr[:, 64:65])
                    nc.vector.tensor_scalar(
                        osb_all[:, j, :], tr[:, 0:64], rc[:], None,
                        op0=mybir.AluOpType.mult)
                nc.scalar.dma_start(
                    out=out_r[:, cq * 4:(cq + 1) * 4, :],
                    in_=osb_all[:, cq * 4:(cq + 1) * 4, :])

    nc.finalize()
    return nc


def get_nc():
    if "nc" not in _CACHE:
        _CACHE["nc"] = _build_nc()
    return _CACHE["nc"]


def _stage_x(x):
    # [S, E] f32 -> [128, NB, EC, CB] f16 with [p, b, c, s] = x[b*CB+s, c*128+p]
    xt = np.ascontiguousarray(x.T.astype(np.float16))          # [E, S]
    xt = xt.reshape(EC, 128, NB, CB).transpose(1, 2, 0, 3)     # [p, b, c, s]
    return np.ascontiguousarray(xt)


def make_in_maps(inputs):
    q = np.asarray(inputs["query"], np.float32)
    k = np.asarray(inputs["key_"], np.float32)
    v = np.asarray(inputs["value"], np.float32)
    wmats = {}
    for nm, key in (("wq", "Wq"), ("wk", "Wk"), ("wv", "Wv")):
        w = np.asarray(inputs[key], np.float32).astype(np.float16)  # [E, H]
        wmats[nm] = np.ascontiguousarray(
            w.reshape(EC, 128, H).transpose(1, 0, 2))               # [128, EC, H]
    bq = np.asarray(inputs["bq"], np.float32).reshape(H, 1)
    bv = np.asarray(inputs["bv"], np.float32).reshape(H, 1)
    bq_d = np.ascontiguousarray(np.tile(bq, (2, 1)))                # [128, 1]
    bv_d = np.ascontiguousarray(np.tile(bv, (2, 1)))
    in_maps = []
    for b in range(B):
        in_maps.append({
            "xq": _stage_x(q[b]),
            "xk": _stage_x(k[b]),
            "xv": _stage_x(v[b]),
            "wq": wmats["wq"], "wk": wmats["wk"], "wv": wmats["wv"],
            "bq": bq_d, "bv": bv_d,
        })
    return in_maps


def kernel(**inputs):
    nc = get_nc()
    in_maps = make_in_maps(inputs)
    res = run_bass_kernel_spmd(nc, in_maps, list(range(B)))
    return np.stack([res.results[b]["out"] for b in range(B)], axis=0)


# revision 41
# speedup vs baseline: 1.3566x; 1.2107x over previous
"""Trainium2 Bass kernel: single attention head (B=8, S=2048, E=1024, H=64).

Sharding: data-parallel over batch -- each of the 8 NeuronCores computes one
batch element's full attention. No collectives; every HBM byte read once.

v9 design (duplicated-weight projections, chunk-major q, 512-col kv blocks):
  - Inputs cast to fp16 HOST-side; HWDGE (sync) block DMAs with contiguous
    per-partition lines. Half the HBM traffic of the f32 original.
  - q projection: the stationary is [Wq | Wq] ([128, 128] per chunk, FWL
    eligible), so ONE matmul per (chunk, 512-col segment) yields q^T in
    BOTH SBUF partition halves -- no column tiling, 8x fewer LDWEIGHTS
    (each weight load costs ~110-180ns of PE time since every matmul
    re-loads its stationary). Chunk-major over resident xq halves.
  - k/v stream in four 512-col blocks; per block one column-tiled k||v
    projection pass (k in partition half (jb%2), v in the other), feeding
    row-tiled scores: tiles from even blocks run in array rows 0-63
    concurrently with odd-block tiles in rows 64-127 into different PSUM
    banks. Scores at N=512.
  - exp on ScalarE over [128, 1024] slabs (32 calls, ~36us total -- the
    global throughput floor). AV matmuls ([v|1]^T @ exp(S^T) into a
    [65, 2048] PSUM accumulator) are queued and used as always-ready
    filler between score groups so the in-order PE FIFO stays dense and
    the HAM clock gate holds 2.4 GHz.
  - Scores stay transposed (keys on partitions); softmax rowsums ride the
    ones column of the AV stationary; bk cancels in softmax; bq/bv fold
    into projection evacuations.
  - Finalize: PE transposes 128-col chunks, VectorE reciprocal + scale,
    batched f32 DMA out.

PSUM: 2 x 2-bank rotating slots (projections + score slabs + finalize
transposes) + 4 banks AV accumulator = 8 banks exactly.
"""

import numpy as np

import concourse.bass as bass  # noqa: F401  (engine namespaces live on nc)
import concourse.mybir as mybir
import concourse.tile as tile
from concourse import bacc
from concourse.bass_utils import run_bass_kernel_spmd
from concourse.masks import make_identity

B, S, E, H = 8, 2048, 1024, 64
EC = E // 128    # contraction chunks (128 partitions each)
KB = 512         # kv block columns
NKB = S // KB    # 4 kv blocks
NT = S // 128    # key tiles
F16 = mybir.dt.float16
F32 = mybir.dt.float32

_CACHE = {}


def _build_nc():
    nc = bacc.Bacc(None)
    xq = nc.declare_dram_parameter("xq", [128, 2, EC, S // 2], F16, isOutput=False)
    xk = nc.declare_dram_parameter("xk", [128, NKB, EC, KB], F16, isOutput=False)
    xv = nc.declare_dram_parameter("xv", [128, NKB, EC, KB], F16, isOutput=False)
    wqd = nc.declare_dram_parameter("wqd", [128, EC, 128], F16, isOutput=False)
    wk = nc.declare_dram_parameter("wk", [128, EC, H], F16, isOutput=False)
    wv = nc.declare_dram_parameter("wv", [128, EC, H], F16, isOutput=False)
    bq = nc.declare_dram_parameter("bq", [128, 1], F32, isOutput=False)
    bv = nc.declare_dram_parameter("bv", [128, 1], F32, isOutput=False)
    out = nc.declare_dram_parameter("out", [S, H], F32, isOutput=True)

    Exp = mybir.ActivationFunctionType.Exp

    with tile.TileContext(nc) as tc:
        with tc.tile_pool(name="const", bufs=1) as const, \
             tc.tile_pool(name="xkp", bufs=3) as xkp, \
             tc.tile_pool(name="xvp", bufs=3) as xvp, \
             tc.tile_pool(name="ptp", bufs=10) as ptp, \
             tc.tile_pool(name="vtp", bufs=2) as vtp, \
             tc.tile_pool(name="p5sb", bufs=2) as p5sb, \
             tc.tile_pool(name="psp", bufs=2, space="PSUM") as psp, \
             tc.tile_pool(name="oap", bufs=1, space="PSUM") as oap:

            # ---- constants ----
            wqd_t = const.tile([128, EC, 128], F16, name="wqd_t")
            nc.sync.dma_start(out=wqd_t[:], in_=wqd[:])
            wk_t = const.tile([128, EC, H], F16, name="wk_t")
            nc.sync.dma_start(out=wk_t[:], in_=wk[:])
            wv_t = const.tile([128, EC, H], F16, name="wv_t")
            nc.sync.dma_start(out=wv_t[:], in_=wv[:])
            bq_t = const.tile([128, 1], F32, name="bq_t")
            nc.sync.dma_start(out=bq_t[:], in_=bq[:])
            bv_t = const.tile([128, 1], F32, name="bv_t")
            nc.sync.dma_start(out=bv_t[:], in_=bv[:])

            qt = const.tile([128, S], F16, name="qt")     # q^T in BOTH halves
            kt = const.tile([128, S], F16, name="kt")     # k^T: half (jb%2)
            xqt = const.tile([128, EC, S], F16, name="xqt")
            vaug = const.tile([128, NT, 80], F16, name="vaug")
            oasb = const.tile([65, S], F16, name="oasb")
            ident = const.tile([128, 128], F16, name="ident")
            osb_all = const.tile([128, NT, H], F32, name="osb_all")

            make_identity(nc, ident[:])
            nc.vector.memset(vaug[:, :, 64], 1.0)

            oa = oap.tile([65, S], F32, name="oa")        # AV accumulator

            def slot(name):
                return psp.tile([128, 1024], F32, tag="ps", name=name)

            # ---- input DMAs (sync HWDGE FIFO) ----
            nc.sync.dma_start(out=xqt[:, :, 0:1024], in_=xq[:, 0])
            xkts, xvts = [], []

            def fetch(which, jb):
                if which == "k":
                    xt = xkp.tile([128, EC, KB], F16, tag="xk", name=f"xkt{jb}")
                    nc.sync.dma_start(out=xt[:], in_=xk[:, jb])
                    xkts.append(xt)
                else:
                    xt = xvp.tile([128, EC, KB], F16, tag="xv", name=f"xvt{jb}")
                    nc.sync.dma_start(out=xt[:], in_=xv[:, jb])
                    xvts.append(xt)

            fetch("k", 0)
            fetch("v", 0)
            fetch("k", 1)
            fetch("v", 1)
            nc.sync.dma_start(out=xqt[:, :, 1024:2048], in_=xq[:, 1])
            fetch("k", 2)
            fetch("v", 2)
            fetch("k", 3)
            fetch("v", 3)

            # ---- PE warm-keeper (before real matmul stream begins) ----
            wslot = slot("warm")
            for _ in range(30):
                nc.tensor.matmul(
                    wslot[0:128, 0:128], ident[:], ident[:],
                    start=True, stop=True, skip_group_check=True)

            # ---- AV queue: always-ready filler matmuls ----
            pts = [None] * NT           # per-tile exp(S^T) SBUF tiles
            av_ready = []
            av_bank_count = [0] * 4

            def emit_av(n):
                while n > 0 and av_ready:
                    t, qh = av_ready.pop(0)
                    for sg in range(2):
                        seg = 2 * qh + sg
                        cnt = av_bank_count[seg]
                        nc.tensor.matmul(
                            oa[:, seg * 512:(seg + 1) * 512],
                            vaug[:, t, 0:65],
                            pts[t][:, seg * 512:(seg + 1) * 512],
                            start=(cnt == 0), stop=(cnt == NT - 1),
                            skip_group_check=True)
                        av_bank_count[seg] = cnt + 1
                    n -= 1

            def qproj(qh):
                ps = slot(f"pq{qh}")
                for c in range(EC):
                    for sg in range(2):
                        nc.tensor.matmul(
                            ps[:, sg * 512:(sg + 1) * 512],
                            wqd_t[:, c, :],
                            xqt[:, c, qh * 1024 + sg * 512:
                                qh * 1024 + (sg + 1) * 512],
                            start=(c == 0), stop=(c == EC - 1),
                            skip_group_check=True)
                nc.vector.tensor_scalar_add(
                    qt[:, qh * 1024:(qh + 1) * 1024], ps[:], bq_t[:])

            def kvproj(jb):
                kh = (jb % 2) * 64
                vh = 64 - kh
                ps = slot(f"pkv{jb}")
                for c in range(EC):
                    nc.tensor.matmul(
                        ps[kh:kh + 64, 0:KB], wk_t[:, c, :], xkts[jb][:, c, :],
                        start=(c == 0), stop=(c == EC - 1),
                        skip_group_check=True)
                    nc.tensor.matmul(
                        ps[vh:vh + 64, 0:KB], wv_t[:, c, :], xvts[jb][:, c, :],
                        start=(c == 0), stop=(c == EC - 1),
                        skip_group_check=True)
                vtb = vtp.tile([128, KB], F16, tag="vt", name=f"vtb{jb}")
                nc.vector.tensor_scalar_add(
                    vtb[vh:vh + 64, :], ps[vh:vh + 64, 0:KB], bv_t[vh:vh + 64])
                nc.sync.dma_start_transpose(
                    vaug[:, 4 * jb:4 * jb + 4, 0:64], vtb[vh:vh + 64, :])
                nc.vector.tensor_copy(
                    kt[kh:kh + 64, jb * KB:(jb + 1) * KB], ps[kh:kh + 64, 0:KB])

            def score_pair(ta, tb_, qh):
                """Row-tiled concurrent scores for tiles ta (rows 0-63) and
                tb_ (rows 64-127), one 1024-col slab + exp each."""
                qs = slice(qh * 1024, (qh + 1) * 1024)
                for t in (ta, tb_):
                    if pts[t] is None:
                        pts[t] = ptp.tile([128, S], F16, tag="pt", name=f"pt{t}")
                ga = ((ta // 4) % 2) * 64
                gb = ((tb_ // 4) % 2) * 64
                sa = slot(f"sa{ta}_{qh}")
                sb_ = slot(f"sb{tb_}_{qh}")
                for seg in range(2):
                    cs = slice(qh * 1024 + seg * 512, qh * 1024 + (seg + 1) * 512)
                    nc.tensor.matmul(
                        sa[:, seg * 512:(seg + 1) * 512],
                        kt[ga:ga + 64, ta * 128:(ta + 1) * 128], qt[ga:ga + 64, cs],
                        start=True, stop=True, skip_group_check=True)
                    nc.tensor.matmul(
                        sb_[:, seg * 512:(seg + 1) * 512],
                        kt[gb:gb + 64, tb_ * 128:(tb_ + 1) * 128], qt[gb:gb + 64, cs],
                        start=True, stop=True, skip_group_check=True)
                nc.scalar.activation(pts[ta][:, qs], sa[:], Exp, scale=0.125)
                nc.scalar.activation(pts[tb_][:, qs], sb_[:], Exp, scale=0.125)
                emit_av(2)
                av_ready.append((ta, qh))
                av_ready.append((tb_, qh))

            # ---- schedule ----
            qproj(0)
            kvproj(0)
            kvproj(1)
            # tiles 0-3 (half 0) pair with 4-7 (half 1), q half 0
            for pi in range(4):
                score_pair(pi, pi + 4, 0)
            qproj(1)
            for pi in range(4):
                score_pair(pi, pi + 4, 1)
            kvproj(2)
            kvproj(3)
            for qh in range(2):
                for pi in range(4):
                    score_pair(8 + pi, 12 + pi, qh)
            emit_av(len(av_ready))

            # ---- finalize: transpose, normalize, store ----
            out_r = out[:].rearrange("(t p) h -> p t h", p=128)
            for cq in range(4):
                nc.vector.tensor_copy(
                    oasb[:, cq * 512:(cq + 1) * 512],
                    oa[:, cq * 512:(cq + 1) * 512])
                for jj in range(4):
                    j = cq * 4 + jj
                    trt = psp.tile([128, 66], F16, tag="ps", name=f"tr{j}")
                    tr = trt[:, 0:65]
                    nc.tensor.transpose(
                        tr, oasb[:, j * 128:(j + 1) * 128], ident[0:65, 0:65])
                    rc = p5sb.tile([128, 1], F32, tag="rc", name=f"rc{j}")
                    nc.vector.reciprocal(rc[:], tr[:, 64:65])
                    nc.vector.tensor_scalar(
                        osb_all[:, j, :], tr[:, 0:64], rc[:], None,
                        op0=mybir.AluOpType.mult)
                nc.scalar.dma_start(
                    out=out_r[:, cq * 4:(cq + 1) * 4, :],
                    in_=osb_all[:, cq * 4:(cq + 1) * 4, :])

    nc.finalize()
    return nc


def get_nc():
    if "nc" not in _CACHE:
        _CACHE["nc"] = _build_nc()
    return _CACHE["nc"]


def _stage_x(x, nblk, cb):
    # [S, E] f32 -> [128, nblk, EC, cb] f16 with [p, b, c, s] = x[b*cb+s, c*128+p]
    xt = np.ascontiguousarray(x.T.astype(np.float16))          # [E, S]
    xt = xt.reshape(EC, 128, nblk, cb).transpose(1, 2, 0, 3)   # [p, b, c, s]
    return np.ascontiguousarray(xt)


def make_in_maps(inputs):
    q = np.asarray(inputs["query"], np.float32)
    k = np.asarray(inputs["key_"], np.float32)
    v = np.asarray(inputs["value"], np.float32)
    wq_h = np.asarray(inputs["Wq"], np.float32).astype(np.float16)
    wqd_h = np.concatenate([wq_h, wq_h], axis=1)                # [E, 128]
    wqd_s = np.ascontiguousarray(
        wqd_h.reshape(EC, 128, 128).transpose(1, 0, 2))         # [128, EC, 128]
    wmats = {}
    for nm, key in (("wk", "Wk"), ("wv", "Wv")):
        w = np.asarray(inputs[key], np.float32).astype(np.float16)
        wmats[nm] = np.ascontiguousarray(
            w.reshape(EC, 128, H).transpose(1, 0, 2))           # [128, EC, H]
    bq = np.asarray(inputs["bq"], np.float32).reshape(H, 1)
    bv = np.asarray(inputs["bv"], np.float32).reshape(H, 1)
    bq_d = np.ascontiguousarray(np.tile(bq, (2, 1)))            # [128, 1]
    bv_d = np.ascontiguousarray(np.tile(bv, (2, 1)))
    in_maps = []
    for b in range(B):
        in_maps.append({
            "xq": _stage_x(q[b], 2, S // 2),
            "xk": _stage_x(k[b], NKB, KB),
            "xv": _stage_x(v[b], NKB, KB),
            "wqd": wqd_s, "wk": wmats["wk"], "wv": wmats["wv"],
            "bq": bq_d, "bv": bv_d,
        })
    return in_maps


def kernel(**inputs):
    nc = get_nc()
    in_maps = make_in_maps(inputs)
    res = run_bass_kernel_spmd(nc, in_maps, list(range(B)))
    return np.stack([res.results[b]["out"] for b in range(B)], axis=0)


# revision 43
# speedup vs baseline: 1.3576x; 1.0008x over previous
"""Trainium2 Bass kernel: single attention head (B=8, S=2048, E=1024, H=64).

Sharding: data-parallel over batch -- each of the 8 NeuronCores computes one
batch element's full attention. No collectives; every HBM byte read once.

v9 design (duplicated-weight projections, chunk-major q, 512-col kv blocks):
  - Inputs cast to fp16 HOST-side; HWDGE (sync) block DMAs with contiguous
    per-partition lines. Half the HBM traffic of the f32 original.
  - q projection: the stationary is [Wq | Wq] ([128, 128] per chunk, FWL
    eligible), so ONE matmul per (chunk, 512-col segment) yields q^T in
    BOTH SBUF partition halves -- no column tiling, 8x fewer LDWEIGHTS
    (each weight load costs ~110-180ns of PE time since every matmul
    re-loads its stationary). Chunk-major over resident xq halves.
  - k/v stream in four 512-col blocks; per block one column-tiled k||v
    projection pass (k in partition half (jb%2), v in the other), feeding
    row-tiled scores: tiles from even blocks run in array rows 0-63
    concurrently with odd-block tiles in rows 64-127 into different PSUM
    banks. Scores at N=512.
  - exp on ScalarE over [128, 1024] slabs (32 calls, ~36us total -- the
    global throughput floor). AV matmuls ([v|1]^T @ exp(S^T) into a
    [65, 2048] PSUM accumulator) are queued and used as always-ready
    filler between score groups so the in-order PE FIFO stays dense and
    the HAM clock gate holds 2.4 GHz.
  - Scores stay transposed (keys on partitions); softmax rowsums ride the
    ones column of the AV stationary; bk cancels in softmax; bq/bv fold
    into projection evacuations.
  - Finalize: PE transposes 128-col chunks, VectorE reciprocal + scale,
    batched f32 DMA out.

PSUM: 2 x 2-bank rotating slots (projections + score slabs + finalize
transposes) + 4 banks AV accumulator = 8 banks exactly.
"""

import numpy as np

import concourse.bass as bass  # noqa: F401  (engine namespaces live on nc)
import concourse.mybir as mybir
import concourse.tile as tile
from concourse import bacc
from concourse.bass_utils import run_bass_kernel_spmd
from concourse.masks import make_identity

B, S, E, H = 8, 2048, 1024, 64
EC = E // 128    # contraction chunks (128 partitions each)
KB = 512         # kv block columns
NKB = S // KB    # 4 kv blocks
NT = S // 128    # key tiles
F16 = mybir.dt.float16
F32 = mybir.dt.float32

_CACHE = {}


def _build_nc():
    nc = bacc.Bacc(None)
    xq = nc.declare_dram_parameter("xq", [128, 2, EC, S // 2], F16, isOutput=False)
    xk = nc.declare_dram_parameter("xk", [128, NKB, EC, KB], F16, isOutput=False)
    xv = nc.declare_dram_parameter("xv", [128, NKB, EC, KB], F16, isOutput=False)
    wqd = nc.declare_dram_parameter("wqd", [128, EC, 128], F16, isOutput=False)
    wk = nc.declare_dram_parameter("wk", [128, EC, H], F16, isOutput=False)
    wv = nc.declare_dram_parameter("wv", [128, EC, H], F16, isOutput=False)
    bq = nc.declare_dram_parameter("bq", [128, 1], F32, isOutput=False)
    bv = nc.declare_dram_parameter("bv", [128, 1], F32, isOutput=False)
    out = nc.declare_dram_parameter("out", [S, H], F32, isOutput=True)

    Exp = mybir.ActivationFunctionType.Exp

    with tile.TileContext(nc) as tc:
        with tc.tile_pool(name="const", bufs=1) as const, \
             tc.tile_pool(name="xkp", bufs=4) as xkp, \
             tc.tile_pool(name="xvp", bufs=4) as xvp, \
             tc.tile_pool(name="ptp", bufs=10) as ptp, \
             tc.tile_pool(name="vtp", bufs=2) as vtp, \
             tc.tile_pool(name="p5sb", bufs=2) as p5sb, \
             tc.tile_pool(name="psp", bufs=2, space="PSUM") as psp, \
             tc.tile_pool(name="oap", bufs=1, space="PSUM") as oap:

            # ---- constants ----
            # weights/biases go on the scalar HWDGE ring so their dispatch
            # cost doesn't head-of-line block the input stream on sync
            wqd_t = const.tile([128, EC, 128], F16, name="wqd_t")
            nc.scalar.dma_start(out=wqd_t[:], in_=wqd[:])
            wk_t = const.tile([128, EC, H], F16, name="wk_t")
            nc.scalar.dma_start(out=wk_t[:], in_=wk[:])
            wv_t = const.tile([128, EC, H], F16, name="wv_t")
            nc.scalar.dma_start(out=wv_t[:], in_=wv[:])
            bq_t = const.tile([128, 1], F32, name="bq_t")
            nc.scalar.dma_start(out=bq_t[:], in_=bq[:])
            bv_t = const.tile([128, 1], F32, name="bv_t")
            nc.scalar.dma_start(out=bv_t[:], in_=bv[:])

            qt = const.tile([128, S], F16, name="qt")     # q^T in BOTH halves
            kt = const.tile([128, S], F16, name="kt")     # k^T: half (jb%2)
            xqt = const.tile([128, EC, S], F16, name="xqt")
            vaug = const.tile([128, NT, 80], F16, name="vaug")
            oasb = const.tile([65, S], F16, name="oasb")
            ident = const.tile([128, 128], F16, name="ident")
            osb_all = const.tile([128, NT, H], F32, name="osb_all")

            make_identity(nc, ident[:])
            nc.vector.memset(vaug[:, :, 64], 1.0)

            oa = oap.tile([65, S], F32, name="oa")        # AV accumulator

            def slot(name):
                return psp.tile([128, 1024], F32, tag="ps", name=name)

            # ---- input DMAs (sync HWDGE FIFO) ----
            nc.sync.dma_start(out=xqt[:, :, 0:1024], in_=xq[:, 0])
            xkts, xvts = [], []

            def fetch(which, jb):
                if which == "k":
                    xt = xkp.tile([128, EC, KB], F16, tag="xk", name=f"xkt{jb}")
                    nc.sync.dma_start(out=xt[:], in_=xk[:, jb])
                    xkts.append(xt)
                else:
                    xt = xvp.tile([128, EC, KB], F16, tag="xv", name=f"xvt{jb}")
                    nc.sync.dma_start(out=xt[:], in_=xv[:, jb])
                    xvts.append(xt)

            fetch("k", 0)
            fetch("v", 0)
            fetch("k", 1)
            fetch("v", 1)
            nc.sync.dma_start(out=xqt[:, :, 1024:2048], in_=xq[:, 1])
            fetch("k", 2)
            fetch("v", 2)
            fetch("k", 3)
            fetch("v", 3)

            # ---- PE warm-keeper (before real matmul stream begins) ----
            wslot = slot("warm")
            for _ in range(30):
                nc.tensor.matmul(
                    wslot[0:128, 0:128], ident[:], ident[:],
                    start=True, stop=True, skip_group_check=True)

            # ---- AV queue: always-ready filler matmuls ----
            pts = [None] * NT           # per-tile exp(S^T) SBUF tiles
            av_ready = []
            av_bank_count = [0] * 4

            def emit_av(n):
                while n > 0 and av_ready:
                    t, qh = av_ready.pop(0)
                    for sg in range(2):
                        seg = 2 * qh + sg
                        cnt = av_bank_count[seg]
                        nc.tensor.matmul(
                            oa[:, seg * 512:(seg + 1) * 512],
                            vaug[:, t, 0:65],
                            pts[t][:, seg * 512:(seg + 1) * 512],
                            start=(cnt == 0), stop=(cnt == NT - 1),
                            skip_group_check=True)
                        av_bank_count[seg] = cnt + 1
                    n -= 1

            def qproj(qh):
                ps = slot(f"pq{qh}")
                for c in range(EC):
                    for sg in range(2):
                        nc.tensor.matmul(
                            ps[:, sg * 512:(sg + 1) * 512],
                            wqd_t[:, c, :],
                            xqt[:, c, qh * 1024 + sg * 512:
                                qh * 1024 + (sg + 1) * 512],
                            start=(c == 0), stop=(c == EC - 1),
                            skip_group_check=True)
                nc.vector.tensor_scalar_add(
                    qt[:, qh * 1024:(qh + 1) * 1024], ps[:], bq_t[:])

            def kvproj(jb):
                kh = (jb % 2) * 64
                vh = 64 - kh
                ps = slot(f"pkv{jb}")
                for c in range(EC):
                    nc.tensor.matmul(
                        ps[kh:kh + 64, 0:KB], wk_t[:, c, :], xkts[jb][:, c, :],
                        start=(c == 0), stop=(c == EC - 1),
                        skip_group_check=True)
                    nc.tensor.matmul(
                        ps[vh:vh + 64, 0:KB], wv_t[:, c, :], xvts[jb][:, c, :],
                        start=(c == 0), stop=(c == EC - 1),
                        skip_group_check=True)
                vtb = vtp.tile([128, KB], F16, tag="vt", name=f"vtb{jb}")
                nc.vector.tensor_scalar_add(
                    vtb[vh:vh + 64, :], ps[vh:vh + 64, 0:KB], bv_t[vh:vh + 64])
                nc.sync.dma_start_transpose(
                    vaug[:, 4 * jb:4 * jb + 4, 0:64], vtb[vh:vh + 64, :])
                nc.vector.tensor_copy(
                    kt[kh:kh + 64, jb * KB:(jb + 1) * KB], ps[kh:kh + 64, 0:KB])

            def score_pair(ta, tb_, qh):
                """Row-tiled concurrent scores for tiles ta (rows 0-63) and
                tb_ (rows 64-127), one 1024-col slab + exp each."""
                qs = slice(qh * 1024, (qh + 1) * 1024)
                for t in (ta, tb_):
                    if pts[t] is None:
                        pts[t] = ptp.tile([128, S], F16, tag="pt", name=f"pt{t}")
                ga = ((ta // 4) % 2) * 64
                gb = ((tb_ // 4) % 2) * 64
                sa = slot(f"sa{ta}_{qh}")
                sb_ = slot(f"sb{tb_}_{qh}")
                for seg in range(2):
                    cs = slice(qh * 1024 + seg * 512, qh * 1024 + (seg + 1) * 512)
                    nc.tensor.matmul(
                        sa[:, seg * 512:(seg + 1) * 512],
                        kt[ga:ga + 64, ta * 128:(ta + 1) * 128], qt[ga:ga + 64, cs],
                        start=True, stop=True, skip_group_check=True)
                    nc.tensor.matmul(
                        sb_[:, seg * 512:(seg + 1) * 512],
                        kt[gb:gb + 64, tb_ * 128:(tb_ + 1) * 128], qt[gb:gb + 64, cs],
                        start=True, stop=True, skip_group_check=True)
                nc.scalar.activation(pts[ta][:, qs], sa[:], Exp, scale=0.125)
                nc.scalar.activation(pts[tb_][:, qs], sb_[:], Exp, scale=0.125)
                emit_av(2)
                av_ready.append((ta, qh))
                av_ready.append((tb_, qh))

            # ---- schedule ----
            qproj(0)
            kvproj(0)
            kvproj(1)
            # tiles 0-3 (half 0) pair with 4-7 (half 1), q half 0
            for pi in range(4):
                score_pair(pi, pi + 4, 0)
            qproj(1)
            for pi in range(4):
                score_pair(pi, pi + 4, 1)
            kvproj(2)
            kvproj(3)
            for qh in range(2):
                for pi in range(4):
                    score_pair(8 + pi, 12 + pi, qh)
            emit_av(len(av_ready))

            # ---- finalize: transpose, normalize, store ----
            out_r = out[:].rearrange("(t p) h -> p t h", p=128)
            for cq in range(4):
                nc.vector.tensor_copy(
                    oasb[:, cq * 512:(cq + 1) * 512],
                    oa[:, cq * 512:(cq + 1) * 512])
                for jj in range(4):
                    j = cq * 4 + jj
                    trt = psp.tile([128, 66], F16, tag="ps", name=f"tr{j}")
                    tr = trt[:, 0:65]
                    nc.tensor.transpose(
                        tr, oasb[:, j * 128:(j + 1) * 128], ident[0:65, 0:65])
                    rc = p5sb.tile([128, 1], F32, tag="rc", name=f"rc{j}")
                    nc.vector.reciprocal(rc[:], tr[:, 64:65])
                    nc.vector.tensor_scalar(
                        osb_all[:, j, :], tr[:, 0:64], rc[:], None,
                        op0=mybir.AluOpType.mult)
                nc.scalar.dma_start(
                    out=out_r[:, cq * 4:(cq + 1) * 4, :],
                    in_=osb_all[:, cq * 4:(cq + 1) * 4, :])

    nc.finalize()
    return nc


def get_nc():
    if "nc" not in _CACHE:
        _CACHE["nc"] = _build_nc()
    return _CACHE["nc"]


def _stage_x(x, nblk, cb):
    # [S, E] f32 -> [128, nblk, EC, cb] f16 with [p, b, c, s] = x[b*cb+s, c*128+p]
    xt = np.ascontiguousarray(x.T.astype(np.float16))          # [E, S]
    xt = xt.reshape(EC, 128, nblk, cb).transpose(1, 2, 0, 3)   # [p, b, c, s]
    return np.ascontiguousarray(xt)


def make_in_maps(inputs):
    q = np.asarray(inputs["query"], np.float32)
    k = np.asarray(inputs["key_"], np.float32)
    v = np.asarray(inputs["value"], np.float32)
    wq_h = np.asarray(inputs["Wq"], np.float32).astype(np.float16)
    wqd_h = np.concatenate([wq_h, wq_h], axis=1)                # [E, 128]
    wqd_s = np.ascontiguousarray(
        wqd_h.reshape(EC, 128, 128).transpose(1, 0, 2))         # [128, EC, 128]
    wmats = {}
    for nm, key in (("wk", "Wk"), ("wv", "Wv")):
        w = np.asarray(inputs[key], np.float32).astype(np.float16)
        wmats[nm] = np.ascontiguousarray(
            w.reshape(EC, 128, H).transpose(1, 0, 2))           # [128, EC, H]
    bq = np.asarray(inputs["bq"], np.float32).reshape(H, 1)
    bv = np.asarray(inputs["bv"], np.float32).reshape(H, 1)
    bq_d = np.ascontiguousarray(np.tile(bq, (2, 1)))            # [128, 1]
    bv_d = np.ascontiguousarray(np.tile(bv, (2, 1)))
    in_maps = []
    for b in range(B):
        in_maps.append({
            "xq": _stage_x(q[b], 2, S // 2),
            "xk": _stage_x(k[b], NKB, KB),
            "xv": _stage_x(v[b], NKB, KB),
            "wqd": wqd_s, "wk": wmats["wk"], "wv": wmats["wv"],
            "bq": bq_d, "bv": bv_d,
        })
    return in_maps


def kernel(**inputs):
    nc = get_nc()
    in_maps = make_in_maps(inputs)
    res = run_bass_kernel_spmd(nc, in_maps, list(range(B)))
    return np.stack([res.results[b]["out"] for b in range(B)], axis=0)


# revision 45
# speedup vs baseline: 1.3843x; 1.0196x over previous
"""Trainium2 Bass kernel: single attention head (B=8, S=2048, E=1024, H=64).

Sharding: data-parallel over batch -- each of the 8 NeuronCores computes one
batch element's full attention. No collectives; every HBM byte read once.

v9 design (duplicated-weight projections, chunk-major q, 512-col kv blocks):
  - Inputs cast to fp16 HOST-side; HWDGE (sync) block DMAs with contiguous
    per-partition lines. Half the HBM traffic of the f32 original.
  - q projection: the stationary is [Wq | Wq] ([128, 128] per chunk, FWL
    eligible), so ONE matmul per (chunk, 512-col segment) yields q^T in
    BOTH SBUF partition halves -- no column tiling, 8x fewer LDWEIGHTS
    (each weight load costs ~110-180ns of PE time since every matmul
    re-loads its stationary). Chunk-major over resident xq halves.
  - k/v stream in four 512-col blocks; per block one column-tiled k||v
    projection pass (k in partition half (jb%2), v in the other), feeding
    row-tiled scores: tiles from even blocks run in array rows 0-63
    concurrently with odd-block tiles in rows 64-127 into different PSUM
    banks. Scores at N=512.
  - exp on ScalarE over [128, 1024] slabs (32 calls, ~36us total -- the
    global throughput floor). AV matmuls ([v|1]^T @ exp(S^T) into a
    [65, 2048] PSUM accumulator) are queued and used as always-ready
    filler between score groups so the in-order PE FIFO stays dense and
    the HAM clock gate holds 2.4 GHz.
  - Scores stay transposed (keys on partitions); softmax rowsums ride the
    ones column of the AV stationary; bk cancels in softmax; bq/bv fold
    into projection evacuations.
  - Finalize: PE transposes 128-col chunks, VectorE reciprocal + scale,
    batched f32 DMA out.

PSUM: 2 x 2-bank rotating slots (projections + score slabs + finalize
transposes) + 4 banks AV accumulator = 8 banks exactly.
"""

import numpy as np

import concourse.bass as bass  # noqa: F401  (engine namespaces live on nc)
import concourse.mybir as mybir
import concourse.tile as tile
from concourse import bacc
from concourse.bass_utils import run_bass_kernel_spmd
from concourse.masks import make_identity

B, S, E, H = 8, 2048, 1024, 64
EC = E // 128    # contraction chunks (128 partitions each)
KB = 512         # kv block columns
NKB = S // KB    # 4 kv blocks
NT = S // 128    # key tiles
F16 = mybir.dt.float16
F32 = mybir.dt.float32

_CACHE = {}


def _build_nc():
    nc = bacc.Bacc(None)
    xq = nc.declare_dram_parameter("xq", [128, 2, EC, S // 2], F16, isOutput=False)
    xk = nc.declare_dram_parameter("xk", [128, NKB, EC, KB], F16, isOutput=False)
    xv = nc.declare_dram_parameter("xv", [128, NKB, EC, KB], F16, isOutput=False)
    wqd = nc.declare_dram_parameter("wqd", [128, EC, 128], F16, isOutput=False)
    wk = nc.declare_dram_parameter("wk", [128, EC, H], F16, isOutput=False)
    wv = nc.declare_dram_parameter("wv", [128, EC, H], F16, isOutput=False)
    bq = nc.declare_dram_parameter("bq", [128, 1], F32, isOutput=False)
    bv = nc.declare_dram_parameter("bv", [128, 1], F32, isOutput=False)
    out = nc.declare_dram_parameter("out", [S, H], F32, isOutput=True)

    Exp = mybir.ActivationFunctionType.Exp

    with tile.TileContext(nc) as tc:
        with tc.tile_pool(name="const", bufs=1) as const, \
             tc.tile_pool(name="xkp", bufs=4) as xkp, \
             tc.tile_pool(name="xvp", bufs=4) as xvp, \
             tc.tile_pool(name="ptp", bufs=10) as ptp, \
             tc.tile_pool(name="vtp", bufs=2) as vtp, \
             tc.tile_pool(name="p5sb", bufs=2) as p5sb, \
             tc.tile_pool(name="psp", bufs=2, space="PSUM") as psp, \
             tc.tile_pool(name="oap", bufs=1, space="PSUM") as oap:

            # ---- constants ----
            # weights/biases go on the scalar HWDGE ring so their dispatch
            # cost doesn't head-of-line block the input stream on sync
            wqd_t = const.tile([128, EC, 128], F16, name="wqd_t")
            nc.scalar.dma_start(out=wqd_t[:], in_=wqd[:])
            wk_t = const.tile([128, EC, H], F16, name="wk_t")
            nc.scalar.dma_start(out=wk_t[:], in_=wk[:])
            wv_t = const.tile([128, EC, H], F16, name="wv_t")
            nc.scalar.dma_start(out=wv_t[:], in_=wv[:])
            bq_t = const.tile([128, 1], F32, name="bq_t")
            nc.scalar.dma_start(out=bq_t[:], in_=bq[:])
            bv_t = const.tile([128, 1], F32, name="bv_t")
            nc.scalar.dma_start(out=bv_t[:], in_=bv[:])

            qt = const.tile([128, S], F16, name="qt")     # q^T in BOTH halves
            kt = const.tile([128, S], F16, name="kt")     # k^T: half (jb%2)
            xqt = const.tile([128, EC, S], F16, name="xqt")
            vaug = const.tile([128, NT, 80], F16, name="vaug")
            oasb = const.tile([65, S], F16, name="oasb")
            ident = const.tile([128, 128], F16, name="ident")
            osb_all = const.tile([128, NT, H], F32, name="osb_all")

            make_identity(nc, ident[:])
            nc.vector.memset(vaug[:, :, 64], 1.0)

            oa = oap.tile([65, S], F32, name="oa")        # AV accumulator

            def slot(name):
                return psp.tile([128, 1024], F32, tag="ps", name=name)

            # ---- input DMAs (sync HWDGE FIFO) ----
            nc.sync.dma_start(out=xqt[:, :, 0:1024], in_=xq[:, 0])
            xkts, xvts = [], []

            def fetch(which, jb):
                if which == "k":
                    xt = xkp.tile([128, EC, KB], F16, tag="xk", name=f"xkt{jb}")
                    nc.sync.dma_start(out=xt[:], in_=xk[:, jb])
                    xkts.append(xt)
                else:
                    xt = xvp.tile([128, EC, KB], F16, tag="xv", name=f"xvt{jb}")
                    nc.sync.dma_start(out=xt[:], in_=xv[:, jb])
                    xvts.append(xt)

            fetch("k", 0)
            fetch("v", 0)
            fetch("k", 1)
            fetch("v", 1)
            nc.sync.dma_start(out=xqt[:, :, 1024:2048], in_=xq[:, 1])
            fetch("k", 2)
            fetch("v", 2)
            fetch("k", 3)
            fetch("v", 3)

            # ---- PE warm-keeper (before real matmul stream begins) ----
            wslot = slot("warm")
            for _ in range(30):
                nc.tensor.matmul(
                    wslot[0:128, 0:128], ident[:], ident[:],
                    start=True, stop=True, skip_group_check=True)

            # ---- AV queue: always-ready filler matmuls ----
            pts = [None] * NT           # per-tile exp(S^T) SBUF tiles
            av_ready = []
            av_bank_count = [0] * 4

            def emit_av(n):
                while n > 0 and av_ready:
                    t, qh = av_ready.pop(0)
                    for sg in range(2):
                        seg = 2 * qh + sg
                        cnt = av_bank_count[seg]
                        nc.tensor.matmul(
                            oa[:, seg * 512:(seg + 1) * 512],
                            vaug[:, t, 0:65],
                            pts[t][:, seg * 512:(seg + 1) * 512],
                            start=(cnt == 0), stop=(cnt == NT - 1),
                            skip_group_check=True)
                        av_bank_count[seg] = cnt + 1
                    n -= 1

            def qproj(qh):
                ps = slot(f"pq{qh}")
                for c in range(EC):
                    for sg in range(2):
                        nc.tensor.matmul(
                            ps[:, sg * 512:(sg + 1) * 512],
                            wqd_t[:, c, :],
                            xqt[:, c, qh * 1024 + sg * 512:
                                qh * 1024 + (sg + 1) * 512],
                            start=(c == 0), stop=(c == EC - 1),
                            skip_group_check=True)
                nc.vector.tensor_scalar_add(
                    qt[:, qh * 1024:(qh + 1) * 1024], ps[:], bq_t[:])

            def kvproj(jb):
                kh = (jb % 2) * 64
                vh = 64 - kh
                ps = slot(f"pkv{jb}")
                for c in range(EC):
                    nc.tensor.matmul(
                        ps[kh:kh + 64, 0:KB], wk_t[:, c, :], xkts[jb][:, c, :],
                        start=(c == 0), stop=(c == EC - 1),
                        skip_group_check=True)
                    nc.tensor.matmul(
                        ps[vh:vh + 64, 0:KB], wv_t[:, c, :], xvts[jb][:, c, :],
                        start=(c == 0), stop=(c == EC - 1),
                        skip_group_check=True)
                vtb = vtp.tile([128, KB], F16, tag="vt", name=f"vtb{jb}")
                nc.vector.tensor_scalar_add(
                    vtb[vh:vh + 64, :], ps[vh:vh + 64, 0:KB], bv_t[vh:vh + 64])
                nc.sync.dma_start_transpose(
                    vaug[:, 4 * jb:4 * jb + 4, 0:64], vtb[vh:vh + 64, :])
                nc.vector.tensor_copy(
                    kt[kh:kh + 64, jb * KB:(jb + 1) * KB], ps[kh:kh + 64, 0:KB])

            def score_slabs(cells):
                """Scores + exp for a list of (tile, qh) cells. Consecutive
                cells with opposite row-group parity run concurrently on the
                PE (row tiling) since their slabs sit in different banks."""
                for t, qh in cells:
                    if pts[t] is None:
                        pts[t] = ptp.tile([128, S], F16, tag="pt", name=f"pt{t}")
                mms, exps = [], []
                for t, qh in cells:
                    g = ((t // 4) % 2) * 64
                    sl = slot(f"s{t}_{qh}")
                    for seg in range(2):
                        cs = slice(qh * 1024 + seg * 512,
                                   qh * 1024 + (seg + 1) * 512)
                        mms.append((sl, seg, g, t, cs))
                    exps.append((t, qh, sl))
                # interleave the two cells' matmuls seg-by-seg for pairing
                if len(cells) == 2:
                    mms = [mms[0], mms[2], mms[1], mms[3]]
                for sl, seg, g, t, cs in mms:
                    nc.tensor.matmul(
                        sl[:, seg * 512:(seg + 1) * 512],
                        kt[g:g + 64, t * 128:(t + 1) * 128], qt[g:g + 64, cs],
                        start=True, stop=True, skip_group_check=True)
                for t, qh, sl in exps:
                    nc.scalar.activation(
                        pts[t][:, qh * 1024:(qh + 1) * 1024], sl[:],
                        Exp, scale=0.125)
                    av_ready.append((t, qh))
                emit_av(len(cells))

            # ---- schedule ----
            qproj(0)
            kvproj(0)
            for t in range(4):                    # unpaired, earliest exp
                score_slabs([(t, 0)])
            kvproj(1)
            qproj(1)
            for t in range(4, 8):                 # unpaired: bridges xqh1 DMA
                score_slabs([(t, 0)])
            for pi in range(4):                   # (h0, h1) pairs, q half 1
                score_slabs([(pi, 1), (pi + 4, 1)])
            kvproj(2)
            kvproj(3)
            for qh in range(2):
                for pi in range(4):
                    score_slabs([(8 + pi, qh), (12 + pi, qh)])
            # flush remaining AV, q-half 0 first so oa banks 0/1 stop early
            av_ready.sort(key=lambda cq: cq[1])
            emit_av(len(av_ready))

            # ---- finalize: transpose, normalize, store ----
            # 4 transposes per PSUM slot, one batched reciprocal per chunk
            out_r = out[:].rearrange("(t p) h -> p t h", p=128)
            for cq in range(4):
                nc.vector.tensor_copy(
                    oasb[:, cq * 512:(cq + 1) * 512],
                    oa[:, cq * 512:(cq + 1) * 512])
                trs = psp.tile([128, 4, 66], F16, tag="ps", name=f"trs{cq}")
                for jj in range(4):
                    j = cq * 4 + jj
                    nc.tensor.transpose(
                        trs[:, jj, 0:65], oasb[:, j * 128:(j + 1) * 128],
                        ident[0:65, 0:65])
                rc = p5sb.tile([128, 4], F32, tag="rc", name=f"rc{cq}")
                nc.vector.reciprocal(rc[:], trs[:, :, 64])
                for jj in range(4):
                    j = cq * 4 + jj
                    nc.vector.tensor_scalar(
                        osb_all[:, j, :], trs[:, jj, 0:64], rc[:, jj:jj + 1],
                        None, op0=mybir.AluOpType.mult)
                nc.scalar.dma_start(
                    out=out_r[:, cq * 4:(cq + 1) * 4, :],
                    in_=osb_all[:, cq * 4:(cq + 1) * 4, :])

    nc.finalize()
    return nc


def get_nc():
    if "nc" not in _CACHE:
        _CACHE["nc"] = _build_nc()
    return _CACHE["nc"]


def _stage_x(x, nblk, cb):
    # [S, E] f32 -> [128, nblk, EC, cb] f16 with [p, b, c, s] = x[b*cb+s, c*128+p]
    xt = np.ascontiguousarray(x.T.astype(np.float16))          # [E, S]
    xt = xt.reshape(EC, 128, nblk, cb).transpose(1, 2, 0, 3)   # [p, b, c, s]
    return np.ascontiguousarray(xt)


def make_in_maps(inputs):
    q = np.asarray(inputs["query"], np.float32)
    k = np.asarray(inputs["key_"], np.float32)
    v = np.asarray(inputs["value"], np.float32)
    wq_h = np.asarray(inputs["Wq"], np.float32).astype(np.float16)
    wqd_h = np.concatenate([wq_h, wq_h], axis=1)                # [E, 128]
    wqd_s = np.ascontiguousarray(
        wqd_h.reshape(EC, 128, 128).transpose(1, 0, 2))         # [128, EC, 128]
    wmats = {}
    for nm, key in (("wk", "Wk"), ("wv", "Wv")):
        w = np.asarray(inputs[key], np.float32).astype(np.float16)
        wmats[nm] = np.ascontiguousarray(
            w.reshape(EC, 128, H).transpose(1, 0, 2))           # [128, EC, H]
    bq = np.asarray(inputs["bq"], np.float32).reshape(H, 1)
    bv = np.asarray(inputs["bv"], np.float32).reshape(H, 1)
    bq_d = np.ascontiguousarray(np.tile(bq, (2, 1)))            # [128, 1]
    bv_d = np.ascontiguousarray(np.tile(bv, (2, 1)))
    in_maps = []
    for b in range(B):
        in_maps.append({
            "xq": _stage_x(q[b], 2, S // 2),
            "xk": _stage_x(k[b], NKB, KB),
            "xv": _stage_x(v[b], NKB, KB),
            "wqd": wqd_s, "wk": wmats["wk"], "wv": wmats["wv"],
            "bq": bq_d, "bv": bv_d,
        })
    return in_maps


def kernel(**inputs):
    nc = get_nc()
    in_maps = make_in_maps(inputs)
    res = run_bass_kernel_spmd(nc, in_maps, list(range(B)))
    return np.stack([res.results[b]["out"] for b in range(B)], axis=0)
